# revision 6
# baseline (speedup 1.0000x reference)
"""DTranNER CRF loss kernel for Trainium2 (8 NeuronCores, data-parallel over batch).

Strategy
--------
Batch (B=256) is sharded 8 ways (32 sentences/core).  Each core computes, for
its sentences:

* pairwise CRF log-partition alpha_pp: a 511-step log-semiring scan over the
  streamed feats_pp [b,t,24,24] matrices, run in *factored linear space*
  (state u = exp(fv - s); scalar log-scale s accumulated lazily every R
  steps).  The scan is split into a forward chain (t=0..255) and a backward
  chain (t=510..256) that meet in the middle -- two independent chains halve
  the sequential-latency wall.
* unary CRF log-partition alpha: same recurrence with the constant
  transitions matrix as stationary weights on the tensor engine; forward and
  backward unary chains are packed into one [64, b] tile so each slot is a
  single matmul + a single vector multiply.
* gold-path scores: operand values host-gathered (pure data movement, like
  the other layout transforms); all arithmetic (sums) happens on device.

Per pairwise step (per chain): DVE bf16 multiply (state broadcast along the
outer free dim via a step-0 AP) + segmented X-reduce; the tensor engine
replicates the fragmented reduce output across the 4 partition groups with
constant 0/1 selector matmuls; ACT copies PSUM->SBUF (folding in the
occasional 1/z renorm scale).  The fp32 HBM stream is exp'ed in bulk on ACT
into bf16 tiles.  Host-side prep is layout-only (slicing / transposing).
"""

import numpy as np
import ml_dtypes
from contextlib import ExitStack

import concourse.bass as bass
import concourse.bacc as bacc
import concourse.tile as tile
from concourse import mybir
from concourse.bass_utils import run_bass_kernel_spmd

FP = mybir.dt.float32
BF = mybir.dt.bfloat16
I32 = mybir.dt.int32

B, T, K = 256, 512, 24
START, STOP = 22, 23
NCORES = 8
N1, N2 = 4, 6  # K = N1*N2 partition/free split

AF = mybir.ActivationFunctionType
ALU = mybir.AluOpType
AX = mybir.AxisListType


class _P:
    """Container for build-time params + pools."""



def _pairwise_step(nc, p, E_slice3, state, scale_ap):
    PP = p.PP
    prod = p.sb.tile([PP, N2, K], BF, tag="prod")
    u_b = state["u"][:, :].unsqueeze(1).broadcast_to([PP, N2, K])
    nc.vector.tensor_tensor(out=prod[:], in0=E_slice3, in1=u_b, op=ALU.mult)
    frag = p.sb.tile([PP, N2], BF, tag="frag")
    with nc.allow_low_precision("bf16 CRF inner state"):
        nc.vector.tensor_reduce(out=frag[:], in_=prod[:], axis=AX.X, op=ALU.add)
    urep_ps = p.ps2.tile([PP, K], FP, tag="urep")
    for k in range(N1):
        nc.tensor.matmul(
            out=urep_ps[:, k * N2 : (k + 1) * N2],
            lhsT=p.selw_sb[:, k * PP : (k + 1) * PP],
            rhs=frag[:], start=True, stop=True,
        )
    # PSUM fp32 -> SBUF bf16 copy on the ACT engine so the next step's DVE
    # multiply gets the 2x half-precision mode; a pending lazy renorm scale
    # folds into the same op's scale operand for free.  (GPSIMD cannot touch
    # PSUM, so ACT does the bounce.)
    u_sb = p.sb.tile([PP, K], BF, tag="u_sb")
    nc.scalar.activation(
        out=u_sb[:], in_=urep_ps[:], func=AF.Copy,
        scale=scale_ap if scale_ap is not None else 1.0,
    )
    state["u"] = u_sb
    state["urep_ps"] = urep_ps


def _pairwise_renorm(nc, p, state, zbuf, slot):
    """z = sum(u) -> zbuf[:, slot]; return 1/z (folded into next copy).
    All the ln() calls happen in one batched pass at the end."""
    PP = p.PP
    nc.vector.tensor_reduce(
        out=zbuf[:, slot : slot + 1], in_=state["u"][:], axis=AX.X, op=ALU.add
    )
    rz = p.sb.tile([PP, 1], FP, tag="rz")
    nc.vector.reciprocal(out=rz[:], in_=zbuf[:, slot : slot + 1])
    return rz[:]


def build_kernel(BC=32, TT=512, TC=32, R=8):
    """Build the per-core Bass program.  BC = sentences per core."""
    PP = BC * N1
    H = TT // 2            # forward pairwise steps (matrices t = 0..H-1)
    HB = TT - 1 - H        # backward pairwise steps (matrices t = TT-2..H)
    UROW = 64              # unary packing: rows 0..K fwd, 32..32+K bwd
    SL = H                 # slots
    NF = N2 * K            # 144
    CP = 3.8               # pairwise exp pre-scale (exp(x-CP))
    CU = 3.8               # unary exp pre-scale
    RW = UROW * BC         # ftp2 row stride

    nc = bacc.Bacc("TRN2", target_bir_lowering=False)
    fppF = nc.dram_tensor("fppF", [BC, N1, H, NF], FP, kind="ExternalInput")
    fppB = nc.dram_tensor("fppB", [BC, N1, HB, NF], FP, kind="ExternalInput")
    winit = nc.dram_tensor("winit", [BC, K], FP, kind="ExternalInput")
    ftp2 = nc.dram_tensor("ftp2", [SL, UROW, BC], FP, kind="ExternalInput")
    eflast = nc.dram_tensor("eflast", [K, BC], FP, kind="ExternalInput")
    transT = nc.dram_tensor("transT", [K, K], FP, kind="ExternalInput")
    transO = nc.dram_tensor("transO", [K, K], FP, kind="ExternalInput")
    gvals = nc.dram_tensor("gvals", [BC, 3 * TT + 4], FP, kind="ExternalInput")
    selw = nc.dram_tensor("selw", [PP, N1 * PP], BF, kind="ExternalInput")
    nll = nc.dram_tensor("nll", [BC], FP, kind="ExternalOutput")
    scr = nc.dram_tensor("scratch", [4, BC], FP)

    p = _P()
    p.PP = PP

    with tile.TileContext(nc) as tc, ExitStack() as ctx:
        p.sb = ctx.enter_context(tc.tile_pool(name="sb", bufs=3))
        p.ps2 = ctx.enter_context(tc.tile_pool(name="ps2", bufs=2, space="PSUM"))
        p.ps1 = ctx.enter_context(tc.tile_pool(name="ps1", bufs=1, space="PSUM"))
        big = ctx.enter_context(tc.tile_pool(name="big", bufs=2))
        ebig = ctx.enter_context(tc.tile_pool(name="ebig", bufs=2))
        per = ctx.enter_context(tc.tile_pool(name="per", bufs=1))
        sb, ps1, ps2 = p.sb, p.ps1, p.ps2

        # ---------------- constants ----------------
        cpb = per.tile([128, 1], FP, tag="cpb")
        nc.vector.memset(cpb[:], -CP)
        cub = per.tile([128, 1], FP, tag="cub")
        nc.vector.memset(cub[:], -CU)
        selw_sb = per.tile([PP, N1 * PP], BF, tag="selw")
        nc.sync.dma_start(out=selw_sb[:], in_=selw[:])
        p.selw_sb = selw_sb

        # Unary stationary weights, block matrix [UROW, UROW]:
        #   rows 0..K,  cols 0..K  : exp(transT)[p, n]   (fwd)
        #   rows 32.., cols 32..   : exp(transO)[n, p]   (bwd)
        uwst1 = per.tile([K, K], FP, tag="uwst1")
        nc.sync.dma_start(out=uwst1[:], in_=transT[:])
        uwst2 = per.tile([UROW, K], FP, tag="uwst2")
        nc.sync.dma_start(out=uwst2[32 : 32 + K, :], in_=transO[:])
        uw = per.tile([UROW, UROW], BF, tag="uw")
        nc.vector.memset(uw[:], 0.0)
        nc.scalar.activation(out=uw[0:K, 0:K], in_=uwst1[:], func=AF.Exp)
        nc.scalar.activation(
            out=uw[32 : 32 + K, 32 : 32 + K], in_=uwst2[32 : 32 + K, :], func=AF.Exp
        )

        uones = per.tile([UROW, 2], BF, tag="uones")
        nc.vector.memset(uones[:], 0.0)
        nc.vector.memset(uones[0:K, 0:1], 1.0)
        nc.vector.memset(uones[32 : 32 + K, 1:2], 1.0)
        # usel [2, UROW]: row 0 selects fwd rows, row 1 selects bwd rows.
        # Row 1 can't be written by compute (start partition 1) -> DMA bounce.
        usel = per.tile([2, UROW], BF, tag="usel")
        nc.vector.memset(usel[:], 0.0)
        nc.vector.memset(usel[0:1, 0:K], 1.0)
        rowB = sb.tile([1, UROW], BF, tag="rowB")
        nc.vector.memset(rowB[:], 0.0)
        nc.vector.memset(rowB[0:1, 32 : 32 + K], 1.0)
        nc.sync.dma_start(out=usel[1:2, :], in_=rowB[:])
        # ones [2,1] to sum the two scale rows at the end
        ones2 = per.tile([2, 1], FP, tag="ones2")
        nc.vector.memset(ones2[:], 1.0)

        tc.strict_bb_all_engine_barrier()

        # ---------------- unary Ef table ----------------
        # eft layout: [UROW, SL*BC], slot g at free offset g*BC
        eft = per.tile([UROW, SL * BC], BF, tag="eft")
        nchunk = 4 if SL >= 4 else 1
        cs2 = SL // nchunk
        cstep = cs2 * BC
        src = ftp2[:, :, :].rearrange("s r j -> r s j")
        for c in range(nchunk):
            ftile = big.tile([UROW, cstep], FP, tag="ftp_in")
            nc.sync.dma_start(
                out=ftile[:].rearrange("p (s j) -> p s j", j=BC),
                in_=src[:, c * cs2 : (c + 1) * cs2, :],
            )
            nc.scalar.activation(
                out=eft[:, c * cstep : (c + 1) * cstep], in_=ftile[:], func=AF.Exp, bias=cub[0:UROW, :]
            )

        # ---------------- state init ----------------
        uf0 = per.tile([PP, K], BF, tag="uf0")
        nc.vector.memset(uf0[:], 0.0)
        nc.vector.memset(uf0[:, START : START + 1], 1.0)
        NRN = (H + R - 1) // R + 1
        zbufF = per.tile([PP, NRN], FP, tag="zbufF")
        nc.vector.memset(zbufF[:], 1.0)
        zbufB = per.tile([PP, NRN], FP, tag="zbufB")
        nc.vector.memset(zbufB[:], 1.0)
        zbufU = per.tile([2, NRN * BC], FP, tag="zbufU")
        nc.vector.memset(zbufU[:], 1.0)
        stF = {"u": uf0}

        wfrag_f = sb.tile([PP, N2], FP, tag="wfrag_f")
        nc.sync.dma_start(
            out=wfrag_f[:], in_=winit[:, :].rearrange("b (p1 p2) -> (b p1) p2", p1=N1)
        )
        wfrag = sb.tile([PP, N2], BF, tag="wfrag")
        nc.scalar.activation(out=wfrag[:], in_=wfrag_f[:], func=AF.Exp, bias=cpb[0:PP, :])
        ub_ps = ps1.tile([PP, K], FP, tag="pmisc")
        for k in range(N1):
            nc.tensor.matmul(
                out=ub_ps[:, k * N2 : (k + 1) * N2],
                lhsT=selw_sb[:, k * PP : (k + 1) * PP],
                rhs=wfrag[:],
                start=True,
                stop=True,
            )
        ub0 = per.tile([PP, K], BF, tag="ub0")
        nc.scalar.activation(out=ub0[:], in_=ub_ps[:], func=AF.Copy)
        stB = {"u": ub0}

        # unary state [UROW, BC]
        us0 = per.tile([UROW, BC], BF, tag="us0")
        nc.vector.memset(us0[:], 0.0)
        row1 = sb.tile([1, BC], BF, tag="row1")
        nc.vector.memset(row1[:], 1.0)
        nc.sync.dma_start(out=us0[START : START + 1, :], in_=row1[:])
        tstop = sb.tile([UROW, 1], FP, tag="tstop")
        nc.sync.dma_start(
            out=tstop[32 : 32 + K, :],
            in_=transO[STOP : STOP + 1, :].rearrange("o k -> k o"),
        )
        tstop_e = sb.tile([UROW, 1], BF, tag="tstop_e")
        nc.scalar.activation(out=tstop_e[32 : 32 + K, :], in_=tstop[32 : 32 + K, :], func=AF.Exp)
        nc.vector.tensor_copy(
            out=us0[32 : 32 + K, :], in_=tstop_e[32 : 32 + K, :].broadcast_to([K, BC])
        )
        stU = us0

        tc.strict_bb_all_engine_barrier()

        # ---------------- gold-path score values (host-gathered operands) ----
        gv = per.tile([BC, 3 * TT + 4], FP, tag="gv")
        nc.sync.dma_start(out=gv[:], in_=gvals[:])

        # ---------------- main streamed loop ----------------
        ntiles = (H + TC - 1) // TC
        rzF = rzB = None
        nF = nB = nU = 0
        for it in range(ntiles):
            t0 = it * TC
            ntF = min(TC, H - t0)
            ntB = min(TC, HB - t0)
            ftile = big.tile([PP, TC * NF], FP, tag="ftileF")
            nc.sync.dma_start(
                out=ftile[:, 0 : ntF * NF],
                in_=fppF[:, :, t0 : t0 + ntF, :].rearrange("b n t f -> (b n) (t f)"),
            )
            eF = ebig.tile([PP, TC * NF], BF, tag="eF")
            nc.scalar.activation(out=eF[:, 0 : ntF * NF], in_=ftile[:, 0 : ntF * NF], func=AF.Exp, bias=cpb[0:PP, :])
            if ntB > 0:
                btile = big.tile([PP, TC * NF], FP, tag="ftileB")
                nc.sync.dma_start(
                    out=btile[:, 0 : ntB * NF],
                    in_=fppB[:, :, t0 : t0 + ntB, :].rearrange("b n t f -> (b n) (t f)"),
                )
                eB = ebig.tile([PP, TC * NF], BF, tag="eB")
                nc.scalar.activation(out=eB[:, 0 : ntB * NF], in_=btile[:, 0 : ntB * NF], func=AF.Exp, bias=cpb[0:PP, :])

            for m in range(ntF):
                eF3 = eF[:, m * NF : (m + 1) * NF].rearrange("q (a b) -> q a b", a=N2)
                _pairwise_step(nc, p, eF3, stF, rzF)
                rzF = None
                nF += 1
                if m < ntB:
                    eB3 = eB[:, m * NF : (m + 1) * NF].rearrange("q (a b) -> q a b", a=N2)
                    _pairwise_step(nc, p, eB3, stB, rzB)
                    rzB = None
                    nB += 1

                # ---- unary slot: mul-first then matvec ----
                g = nU
                ef_sl = eft[:, g * BC : (g + 1) * BC]
                us_m = sb.tile([UROW, BC], BF, tag="us_m")
                nc.vector.tensor_tensor(out=us_m[:], in0=stU[:], in1=ef_sl, op=ALU.mult)
                vu_ps = ps2.tile([UROW, BC], FP, tag="vu")
                nc.tensor.matmul(out=vu_ps[:], lhsT=uw[:], rhs=us_m[:], start=True, stop=True)
                stU = vu_ps
                nU += 1

                # ---- lazy renorms ----
                if nF % R == 0 and nF < H:
                    rzF = _pairwise_renorm(nc, p, stF, zbufF, nF // R)
                if nB > 0 and nB % R == 0 and nB < HB and m < ntB:
                    rzB = _pairwise_renorm(nc, p, stB, zbufB, nB // R)
                if nU % R == 0 and nU < SL:
                    us_c = sb.tile([UROW, BC], BF, tag="us_c")
                    nc.scalar.activation(out=us_c[:], in_=stU[:], func=AF.Copy)
                    stU = us_c
                    zu_ps = ps1.tile([2, BC], FP, tag="pmisc")
                    nc.tensor.matmul(out=zu_ps[:], lhsT=uones[:], rhs=stU[:], start=True, stop=True)
                    zsl = zbufU[:, (nU // R) * BC : (nU // R + 1) * BC]
                    nc.vector.tensor_copy(out=zsl, in_=zu_ps[:])
                    rzu = sb.tile([2, BC], FP, tag="rzu")
                    nc.vector.reciprocal(out=rzu[:], in_=zu_ps[:])
                    rzu_b = sb.tile([2, BC], BF, tag="rzu_b")
                    nc.vector.tensor_copy(out=rzu_b[:], in_=rzu[:])
                    rzu_rep = ps1.tile([UROW, BC], FP, tag="pmisc")
                    nc.tensor.matmul(out=rzu_rep[:], lhsT=usel[:], rhs=rzu_b[:], start=True, stop=True)
                    rzu_s = sb.tile([UROW, BC], BF, tag="rzu_s")
                    nc.scalar.activation(out=rzu_s[:], in_=rzu_rep[:], func=AF.Copy)
                    us_sc = sb.tile([UROW, BC], BF, tag="us_s")
                    nc.vector.tensor_tensor(out=us_sc[:], in0=stU[:], in1=rzu_s[:], op=ALU.mult)
                    stU = us_sc

        # ---------------- tails ----------------
        # batched ln of all buffered renorm z values, then sum per chain
        lzF = sb.tile([PP, NRN], FP, tag="lzF")
        nc.scalar.activation(out=lzF[:], in_=zbufF[:], func=AF.Ln)
        sF = sb.tile([PP, 1], FP, tag="sF")
        nc.vector.tensor_reduce(out=sF[:], in_=lzF[:], axis=AX.X, op=ALU.add)
        lzB = sb.tile([PP, NRN], FP, tag="lzB")
        nc.scalar.activation(out=lzB[:], in_=zbufB[:], func=AF.Ln)
        sB = sb.tile([PP, 1], FP, tag="sB")
        nc.vector.tensor_reduce(out=sB[:], in_=lzB[:], axis=AX.X, op=ALU.add)
        lzU = sb.tile([2, NRN * BC], FP, tag="lzU")
        nc.scalar.activation(out=lzU[:], in_=zbufU[:], func=AF.Ln)
        sU = sb.tile([2, BC], FP, tag="sU")
        nc.vector.tensor_reduce(
            out=sU[:],
            in_=lzU[:].rearrange("a (s b) -> a b s", b=BC),
            axis=AX.X,
            op=ALU.add,
        )
        # pairwise meet
        ufc = sb.tile([PP, K], BF, tag="ufc")
        nc.scalar.activation(out=ufc[:], in_=stF["u"][:], func=AF.Copy)
        pm = sb.tile([PP, K], FP, tag="pmeet")
        nc.vector.tensor_tensor(out=pm[:], in0=ufc[:], in1=stB["u"][:], op=ALU.mult)
        qq = sb.tile([PP, 1], FP, tag="qq")
        nc.vector.tensor_reduce(out=qq[:], in_=pm[:], axis=AX.X, op=ALU.add)
        lq = sb.tile([PP, 1], FP, tag="lq")
        nc.scalar.activation(out=lq[:], in_=qq[:], func=AF.Ln)
        nc.vector.tensor_tensor(out=lq[:], in0=lq[:], in1=sF[:], op=ALU.add)
        nc.vector.tensor_tensor(out=lq[:], in0=lq[:], in1=sB[:], op=ALU.add)
        nc.vector.tensor_scalar(out=lq[:], in0=lq[:], scalar1=CP * (H + HB + 1), scalar2=None, op0=ALU.add)
        nc.sync.dma_start(
            out=scr[0:1, :],
            in_=lq[:, :].rearrange("(b n) o -> b (n o)", n=N1)[:, 0:1],
        )

        # unary meet (incl. deferred Ef_{H-1} diag factor)
        efl = sb.tile([K, BC], FP, tag="efl")
        nc.sync.dma_start(out=efl[:], in_=eflast[:])
        efl_e = sb.tile([K, BC], BF, tag="efl_e")
        nc.scalar.activation(out=efl_e[:], in_=efl[:], func=AF.Exp)
        ustail = sb.tile([UROW, BC], BF, tag="ustail")
        nc.scalar.activation(out=ustail[:], in_=stU[:], func=AF.Copy)
        stU = ustail
        usb_c = sb.tile([K, BC], BF, tag="usb_c")
        nc.sync.dma_start(out=usb_c[:], in_=stU[32 : 32 + K, :])
        um = sb.tile([K, BC], BF, tag="umeet")
        nc.vector.tensor_tensor(out=um[:], in0=stU[0:K, :], in1=usb_c[:], op=ALU.mult)
        nc.vector.tensor_tensor(out=um[:], in0=um[:], in1=efl_e[:], op=ALU.mult)
        ones_k = sb.tile([K, 1], BF, tag="ones_k")
        nc.vector.memset(ones_k[:], 1.0)
        au_ps = ps1.tile([1, BC], FP, tag="pmisc")
        nc.tensor.matmul(out=au_ps[:], lhsT=ones_k[:], rhs=um[:], start=True, stop=True)
        lau = sb.tile([1, BC], FP, tag="lau")
        nc.scalar.activation(out=lau[:], in_=au_ps[:], func=AF.Ln)
        su_ps = ps1.tile([1, BC], FP, tag="pmisc")
        nc.tensor.matmul(out=su_ps[:], lhsT=ones2[:], rhs=sU[:], start=True, stop=True)
        nc.vector.tensor_tensor(out=lau[:], in0=lau[:], in1=su_ps[:], op=ALU.add)
        nc.vector.tensor_scalar(out=lau[:], in0=lau[:], scalar1=CU * (2 * SL), scalar2=None, op0=ALU.add)
        nc.sync.dma_start(out=scr[1:2, :], in_=lau[:])

        # score reduction (single fused sum of all gold-path terms)
        sc = sb.tile([BC, 1], FP, tag="sc")
        nc.vector.tensor_reduce(out=sc[:], in_=gv[:], axis=AX.X, op=ALU.add)

        app = sb.tile([BC, 1], FP, tag="app")
        nc.sync.dma_start(out=app[:], in_=scr[0:1, :].rearrange("o b -> b o"))
        alu_ = sb.tile([BC, 1], FP, tag="alu")
        nc.sync.dma_start(out=alu_[:], in_=scr[1:2, :].rearrange("o b -> b o"))

        res = sb.tile([BC, 1], FP, tag="res")
        nc.vector.tensor_tensor(out=res[:], in0=app[:], in1=alu_[:], op=ALU.add)
        nc.vector.tensor_tensor(out=res[:], in0=res[:], in1=sc[:], op=ALU.subtract)
        nc.sync.dma_start(out=nll[:], in_=res[:].rearrange("b o -> (b o)"))

    nc.compile()
    return nc


# ======================= host-side prep =======================

def prep_core_inputs(feats, fpp, transitions, tags, b0, BC, TT):
    """Build the per-core input map (pure layout transforms)."""
    H = TT // 2
    HB = TT - 1 - H
    fe = feats[b0 : b0 + BC]          # [BC, T, K]
    fp = fpp[b0 : b0 + BC]            # [BC, T, K*K]
    tg = tags[b0 : b0 + BC]           # [BC, T]
    fp4 = fp.reshape(BC, TT, K, K)    # [b, t, n, p]

    fwd = fp4[:, 0:H].reshape(BC, H, N1, N2, K).transpose(0, 2, 1, 3, 4)
    fppF = np.ascontiguousarray(fwd.reshape(BC, N1, H, N2 * K), np.float32)

    # bwd slot s holds matrix t = TT-2-s, (p-major) transposed
    bwd_t = fp4[:, H : TT - 1][:, ::-1]            # [b, s, n, p]
    bwd = bwd_t.transpose(0, 1, 3, 2)              # [b, s, p, n]
    bwd = bwd.reshape(BC, HB, N1, N2, K).transpose(0, 2, 1, 3, 4)
    fppB = np.ascontiguousarray(bwd.reshape(BC, N1, HB, N2 * K), np.float32)

    winit = np.ascontiguousarray(fp4[:, TT - 1, STOP, :], np.float32)

    # unary Ef table: fwd rows at slot s hold feats[t=s-1] (slot 0 = zeros);
    # bwd rows at slot s hold feats[t=TT-1-s]
    ftp2 = np.zeros((H, 64, BC), np.float32)
    ftp2[1:, 0:K, :] = fe[:, 0 : H - 1].transpose(1, 2, 0)
    ftp2[:, 32 : 32 + K, :] = fe[:, TT - 1 : H - 1 : -1].transpose(1, 2, 0)
    eflast = np.ascontiguousarray(fe[:, H - 1, :].T, np.float32)  # [K, BC]

    # gold-path score operands (gather = data movement; summation on device)
    tgi = np.asarray(tg, np.int64)
    te = np.concatenate([np.full((BC, 1), START, np.int64), tgi,
                         np.full((BC, 1), STOP, np.int64)], axis=1)  # [BC, TT+2]
    nxt, prv = te[:, 1:], te[:, :-1]                                  # [BC, TT+1]
    b_ = np.arange(BC)[:, None]
    t_ = np.arange(TT)[None, :]
    gvals = np.zeros((BC, 3 * TT + 4), np.float32)
    gvals[:, 0 : TT + 1] = transitions[nxt, prv]
    gvals[:, TT + 1 : 2 * TT + 1] = np.take_along_axis(
        fe, tgi[:, :, None], axis=2)[..., 0]
    gvals[:, 2 * TT + 1 : 3 * TT + 1] = fp4[b_, np.minimum(t_, TT - 2),
                                            nxt[:, 0:TT], prv[:, 0:TT]]
    # overwrite the t = TT-1 pp term with the terminal fpp[., TT-1, STOP, tags[-1]]
    gvals[:, 3 * TT] = fp4[np.arange(BC), TT - 1, STOP, tgi[:, -1]]
    gvals[:, 3 * TT - 1] = fp4[np.arange(BC), TT - 2, nxt[:, TT - 2], prv[:, TT - 2]]

    PP = BC * N1
    selw = np.zeros((PP, N1, PP), np.float32)
    b_idx = np.arange(BC)
    for k in range(N1):
        for n1p in range(N1):
            selw[b_idx * N1 + k, k, b_idx * N1 + n1p] = 1.0
    selw = selw.reshape(PP, N1 * PP).astype(ml_dtypes.bfloat16)

    return {
        "fppF": fppF,
        "fppB": fppB,
        "winit": winit,
        "ftp2": ftp2,
        "eflast": eflast,
        "transT": np.ascontiguousarray(transitions.T, np.float32),
        "transO": np.ascontiguousarray(transitions, np.float32),
        "gvals": gvals,
        "selw": selw,
    }


_NC_CACHE = {}


def get_nc(BC, TT, TC=32, R=8):
    key = (BC, TT, TC, R)
    if key not in _NC_CACHE:
        _NC_CACHE[key] = build_kernel(BC=BC, TT=TT, TC=TC, R=R)
    return _NC_CACHE[key]


def kernel(feats, feats_pp, transitions, tags):
    feats = np.asarray(feats, np.float32)
    feats_pp = np.asarray(feats_pp, np.float32)
    transitions = np.asarray(transitions, np.float32)
    tags_np = np.asarray(tags)

    BC = B // NCORES
    nc = get_nc(BC, T)
    in_maps = [
        prep_core_inputs(feats, feats_pp, transitions, tags_np, c * BC, BC, T)
        for c in range(NCORES)
    ]
    r = run_bass_kernel_spmd(nc, in_maps, list(range(NCORES)))
    out = np.concatenate([r.results[c]["nll"] for c in range(NCORES)])
    return out.astype(np.float32)


if __name__ == "__main__":
    rng = np.random.default_rng(0)
    feats = rng.standard_normal((B, T, K), dtype=np.float32)
    fpp = rng.standard_normal((B, T, K * K), dtype=np.float32)
    tr = rng.standard_normal((K, K), dtype=np.float32)
    tr[START, :] = -100.0
    tr[:, STOP] = -100.0
    tags = rng.integers(0, K - 2, size=(B, T)).astype(np.int32)
    out = kernel(feats, fpp, tr, tags)
    print(out.shape, out[:4])



# revision 8
# speedup vs baseline: 1.3754x; 1.3754x over previous
"""DTranNER CRF loss kernel for Trainium2 — v2: PE-resident pairwise scan.

Batch (B=256) is sharded 8 ways (32 sentences/core).  The 511-step pairwise
log-semiring scan runs entirely on the tensor engine in factored linear
space, split into four concurrent chain families so no sequential chain
exceeds 192 steps:

* forward vector chain  (t = 0..191):   v <- M_t v,  one [24,24]@[24,1]
  matmul per (sentence, step); lhsT = exp(fpp_t)^T streamed from HBM.
* backward vector chain (t = 510..319): r <- M_t^T r (natural-layout lhsT).
* two interior chunk operators (t = 192..255, 256..318+pad): 24-column
  basis propagation S <- M_t S, one [24,24]@[24,24] matmul per step.
* stitch: alpha_pp = ln( (S_1^T (S_2^T r)) . v ) + const, with constant
  log-shifts between stages (no per-lane renorms needed at these depths).

Lanes are packed 3-per-128-partitions (PE operands must sit at partition
bases 0/32/64); all per-step PSUM->SBUF state copies are batched into 1-2
DVE ops per step.  The fp32 stream is exp'ed in bulk on ACT one tile ahead
of use; interior-chunk stream DMAs issue from the (otherwise idle) GPSIMD
queue so the SP queue serves only the end-chain stream -- the two DMA
pipelines then never head-of-line block each other.  The unary CRF chain
(constant-transition matmuls on PE + DVE multiplies, lazy renorm every 32
slots, all renorm ops on DVE/PE so the ACT exp pipeline is never in its
path) and the gold-path score reduction are as in the DVE baseline.
"""

import numpy as np
import ml_dtypes
from contextlib import ExitStack

import concourse.bass as bass
import concourse.bacc as bacc
import concourse.tile as tile
from concourse import mybir
from concourse.bass_utils import run_bass_kernel_spmd

FP = mybir.dt.float32
BF = mybir.dt.bfloat16

B, T, K = 256, 512, 24
START, STOP = 22, 23
NCORES = 8

AF = mybir.ActivationFunctionType
ALU = mybir.AluOpType
AX = mybir.AxisListType

# chain partition of the 511 chain matrices (t = 0..510)
LE = 192          # fwd covers t [0,192), bwd covers t [319,511) descending
NI = 2            # interior chunks
LI = 64           # interior chunk length (incl. 1 identity pad step)
GQ = 11           # lane column-blocks per 32-sentence family (ceil(32/3))
GE = 2 * GQ       # end-chain column blocks (fwd 0..10, bwd 11..21)
GI = NI * GQ      # interior column blocks
TCS = 4           # steps per streamed tile
CP = 3.8          # exp pre-scale: matrices enter as exp(x - CP)
CU = 3.8          # unary exp pre-scale
BIAS1 = 30.0      # stitch stage-1 log-shift
BIAS2 = 8.0       # stitch stage-2 log-shift
UROW = 64
SL = 256          # unary slots (fwd+bwd packed)
R = 32            # unary lazy-renorm cadence


BUFS_BIG = 2
BUFS_EBIG = 2
PACE_NS = 0
DO_UNARY = True
DO_PAIR = True
DO_ENDS = True
DO_INT = True


def build_kernel(BC=32):
    assert BC == 32
    NTE = LE // TCS          # 24 end steptiles
    NTI = LI // TCS          # 8 interior steptiles
    CWE = GE * TCS * K       # 4224 cols per end steptile
    CWI = GI * TCS * K       # 4224 cols per interior steptile

    nc = bacc.Bacc("TRN2", target_bir_lowering=False)
    fppE = nc.dram_tensor("fppE", [NTE, 3, K, GE, TCS, K], FP, kind="ExternalInput")
    fppI = nc.dram_tensor("fppI", [NTI, 3, K, GI, TCS, K], FP, kind="ExternalInput")
    endS0 = nc.dram_tensor("endS0", [128, GE], BF, kind="ExternalInput")
    identI = nc.dram_tensor("identI", [128, GI * K], BF, kind="ExternalInput")
    ftp2 = nc.dram_tensor("ftp2", [SL, UROW, BC], FP, kind="ExternalInput")
    eflast = nc.dram_tensor("eflast", [K, BC], FP, kind="ExternalInput")
    transT = nc.dram_tensor("transT", [K, K], FP, kind="ExternalInput")
    transO = nc.dram_tensor("transO", [K, K], FP, kind="ExternalInput")
    gvals = nc.dram_tensor("gvals", [BC, 3 * T + 4], FP, kind="ExternalInput")
    nll = nc.dram_tensor("nll", [BC], FP, kind="ExternalOutput")
    scr = nc.dram_tensor("scratch", [4, 40], FP)

    with tile.TileContext(nc) as tc, ExitStack() as ctx:
        sb = ctx.enter_context(tc.tile_pool(name="sb", bufs=3))
        ps2 = ctx.enter_context(tc.tile_pool(name="ps2", bufs=2, space="PSUM"))
        ps1 = ctx.enter_context(tc.tile_pool(name="ps1", bufs=1, space="PSUM"))
        psc = ctx.enter_context(tc.tile_pool(name="psc", bufs=1, space="PSUM"))
        big = ctx.enter_context(tc.tile_pool(name="big", bufs=BUFS_BIG))
        ebig = ctx.enter_context(tc.tile_pool(name="ebig", bufs=BUFS_EBIG))
        per = ctx.enter_context(tc.tile_pool(name="per", bufs=1))

        # ---------------- constants ----------------
        cpb = per.tile([128, 1], FP, tag="cpb")
        nc.vector.memset(cpb[:], -CP)
        cub = per.tile([128, 1], FP, tag="cub")
        nc.vector.memset(cub[:], -CU)

        # unary stationary weights (block matrix, fwd rows 0..K / bwd 32..32+K)
        uwst1 = per.tile([K, K], FP, tag="uwst1")
        nc.sync.dma_start(out=uwst1[:], in_=transT[:])
        uwst2 = per.tile([UROW, K], FP, tag="uwst2")
        nc.sync.dma_start(out=uwst2[32 : 32 + K, :], in_=transO[:])
        uw = per.tile([UROW, UROW], BF, tag="uw")
        nc.vector.memset(uw[:], 0.0)
        nc.scalar.activation(out=uw[0:K, 0:K], in_=uwst1[:], func=AF.Exp)
        nc.scalar.activation(
            out=uw[32 : 32 + K, 32 : 32 + K], in_=uwst2[32 : 32 + K, :], func=AF.Exp
        )
        uones = per.tile([UROW, 2], BF, tag="uones")
        nc.vector.memset(uones[:], 0.0)
        nc.vector.memset(uones[0:K, 0:1], 1.0)
        nc.vector.memset(uones[32 : 32 + K, 1:2], 1.0)
        usel = per.tile([2, UROW], BF, tag="usel")
        nc.vector.memset(usel[:], 0.0)
        nc.vector.memset(usel[0:1, 0:K], 1.0)
        rowB = sb.tile([1, UROW], BF, tag="rowB")
        nc.vector.memset(rowB[:], 0.0)
        nc.vector.memset(rowB[0:1, 32 : 32 + K], 1.0)
        nc.sync.dma_start(out=usel[1:2, :], in_=rowB[:])
        ones2 = per.tile([2, 1], FP, tag="ones2")
        nc.vector.memset(ones2[:], 1.0)
        # quadrant block-ones [128, 3] for the final cross-partition dot
        blk3 = per.tile([128, 3], BF, tag="blk3")
        nc.vector.memset(blk3[:], 0.0)
        for l in range(3):
            nc.vector.memset(blk3[32 * l : 32 * l + K, l : l + 1], 1.0)

        # ---------------- chain states ----------------
        endS = per.tile([128, GE], BF, tag="endS")
        nc.vector.memset(endS[:], 0.0)
        nc.sync.dma_start(out=endS[:], in_=endS0[:])
        intS = per.tile([128, GI * K], BF, tag="intS")
        nc.vector.memset(intS[:], 0.0)
        nc.sync.dma_start(out=intS[:], in_=identI[:])

        endPS = psc.tile([128, GE], FP, tag="endPS")
        nc.vector.memset(endPS[:], 0.0)
        intPSA = psc.tile([128, GQ * K], FP, tag="intPSA")
        nc.vector.memset(intPSA[:], 1.0)
        intPSB = psc.tile([128, GQ * K], FP, tag="intPSB")
        nc.vector.memset(intPSB[:], 1.0)

        tc.strict_bb_all_engine_barrier()

        # ---------------- unary Ef table ----------------
        eft = per.tile([UROW, SL * BC], BF, tag="eft")
        cs2 = SL // 4
        cstep = cs2 * BC
        src = ftp2[:, :, :].rearrange("s r j -> r s j")
        for c in range(4):
            ftile = big.tile([UROW, cstep], FP, tag="ftp_in")
            nc.sync.dma_start(
                out=ftile[:].rearrange("p (s j) -> p s j", j=BC),
                in_=src[:, c * cs2 : (c + 1) * cs2, :],
            )
            nc.scalar.activation(
                out=eft[:, c * cstep : (c + 1) * cstep], in_=ftile[:], func=AF.Exp,
                bias=cub[0:UROW, :],
            )

        # unary state [UROW, BC]
        us0 = per.tile([UROW, BC], BF, tag="us0")
        nc.vector.memset(us0[:], 0.0)
        row1 = sb.tile([1, BC], BF, tag="row1")
        nc.vector.memset(row1[:], 1.0)
        nc.sync.dma_start(out=us0[START : START + 1, :], in_=row1[:])
        tstop = sb.tile([UROW, 1], FP, tag="tstop")
        nc.sync.dma_start(
            out=tstop[32 : 32 + K, :],
            in_=transO[STOP : STOP + 1, :].rearrange("o k -> k o"),
        )
        tstop_e = sb.tile([UROW, 1], BF, tag="tstop_e")
        nc.scalar.activation(out=tstop_e[32 : 32 + K, :], in_=tstop[32 : 32 + K, :], func=AF.Exp)
        nc.vector.tensor_copy(
            out=us0[32 : 32 + K, :], in_=tstop_e[32 : 32 + K, :].broadcast_to([K, BC])
        )
        stU = us0
        NRN = SL // R + 1
        zbufU = per.tile([2, NRN * BC], FP, tag="zbufU")
        nc.vector.memset(zbufU[:], 1.0)

        # gold-path score operands
        gv = per.tile([BC, 3 * T + 4], FP, tag="gv")
        nc.sync.dma_start(out=gv[:], in_=gvals[:])

        tc.strict_bb_all_engine_barrier()

        # ---------------- streamed prefetch helpers ----------------
        def load_end(nt):
            st = big.tile([128, CWE], FP, tag="stageE")
            for l in range(3):
                nc.sync.dma_start(
                    out=st[32 * l : 32 * l + K, :].rearrange(
                        "p (g s n) -> p g s n", g=GE, s=TCS
                    ),
                    in_=fppE[nt, l],
                )
            return st

        def load_int(nt):
            st = big.tile([128, CWI], FP, tag="stageI")
            for l in range(3):
                nc.gpsimd.dma_start(
                    out=st[32 * l : 32 * l + K, :].rearrange(
                        "p (g s n) -> p g s n", g=GI, s=TCS
                    ),
                    in_=fppI[nt, l],
                )
            return st

        # Software pipeline: at the boundary of tile nt, tile nt+1 is already
        # exp'ed and tile nt+2's DMA is in flight — the matmul stream never
        # waits on ACT or HBM.
        def exp_tile(stage, w, tag):
            e = ebig.tile([128, w], BF, tag=tag)
            nc.scalar.activation(out=e[:], in_=stage[:], func=AF.Exp, bias=cpb[:, :])
            return e

        stageE_t = load_end(0)
        stageI_t = load_int(0)
        expE = exp_tile(stageE_t, CWE, "expE")
        expI = exp_tile(stageI_t, CWI, "expI")
        stageE_t = load_end(1)
        stageI_t = load_int(1)
        expE_nxt = expI_nxt = None
        nU = 0

        # ---------------- main loop ----------------
        for s in range(LE):
            if PACE_NS:
                tc.tile_set_cur_wait(s * PACE_NS * 1e-6)
            if s % TCS == 0:
                nt = s // TCS
                if expE_nxt is not None:
                    expE = expE_nxt
                if nt + 1 < NTE:
                    expE_nxt = exp_tile(stageE_t, CWE, "expE")
                    if nt + 2 < NTE:
                        stageE_t = load_end(nt + 2)
            if s % 3 == 0 and s // 3 < LI and (s // 3) % TCS == 0:
                if expI_nxt is not None:
                    expI = expI_nxt
            if s % (3 * TCS) == (3 * TCS) // 2:
                j = s // (3 * TCS)
                if j + 1 < NTI:
                    expI_nxt = exp_tile(stageI_t, CWI, "expI")
                    if j + 2 < NTI:
                        stageI_t = load_int(j + 2)

            # ---- PE: interior chunk matmuls (every 3rd step) ----
            if DO_PAIR and s % 3 == 0 and s // 3 < LI:
                i_s = s // 3
                so = i_s % TCS
                for k in range(NI):
                    for b in range(BC):
                        l, g0 = b % 3, b // 3
                        g = GQ * k + g0
                        pb = 32 * l
                        lhsT = expI[pb : pb + K, (g * TCS + so) * K : (g * TCS + so + 1) * K]
                        if g < GQ:
                            dst = intPSA
                            co = g * K
                        else:
                            dst = intPSB
                            co = (g - GQ) * K
                        nc.tensor.matmul(
                            out=dst[pb : pb + K, co : co + K],
                            lhsT=lhsT,
                            rhs=intS[pb : pb + K, g * K : (g + 1) * K],
                            start=True, stop=True,
                        )

            # ---- PE: end-chain matmuls ----
            so = s % TCS
            for b in (range(BC) if (DO_PAIR and DO_ENDS) else []):
                l, g0 = b % 3, b // 3
                pb = 32 * l
                lhsT = expE[pb : pb + K, (g0 * TCS + so) * K : (g0 * TCS + so + 1) * K]
                nc.tensor.matmul(
                    out=endPS[pb : pb + K, g0 : g0 + 1], lhsT=lhsT,
                    rhs=endS[pb : pb + K, g0 : g0 + 1], start=True, stop=True,
                )
                gB = GQ + g0
                lhsT2 = expE[pb : pb + K, (gB * TCS + so) * K : (gB * TCS + so + 1) * K]
                nc.tensor.matmul(
                    out=endPS[pb : pb + K, gB : gB + 1], lhsT=lhsT2,
                    rhs=endS[pb : pb + K, gB : gB + 1], start=True, stop=True,
                )

            # ---- state copies (PSUM -> SBUF bf16), ahead of the unary ops in
            # the DVE stream so the pairwise chains never queue behind them --
            if DO_PAIR and DO_INT and s % 3 == 0 and s // 3 < LI:
                nc.vector.tensor_copy(out=intS[:, 0 : GQ * K], in_=intPSA[:])
                nc.vector.tensor_copy(out=intS[:, GQ * K : GI * K], in_=intPSB[:])
            if DO_PAIR and DO_ENDS:
                nc.vector.tensor_copy(out=endS[:], in_=endPS[:])

            # ---- unary slots (DVE + PE stationary matmul) ----
            tgt = (s + 1) * SL // LE if DO_UNARY else 0
            while nU < tgt:
                g = nU
                ef_sl = eft[:, g * BC : (g + 1) * BC]
                us_m = sb.tile([UROW, BC], BF, tag="us_m")
                nc.vector.tensor_tensor(out=us_m[:], in0=stU[:], in1=ef_sl, op=ALU.mult)
                vu_ps = ps2.tile([UROW, BC], FP, tag="vu")
                nc.tensor.matmul(out=vu_ps[:], lhsT=uw[:], rhs=us_m[:], start=True, stop=True)
                stU = vu_ps
                nU += 1
                if nU % R == 0 and nU < SL:
                    us_c = sb.tile([UROW, BC], BF, tag="us_c")
                    nc.vector.tensor_copy(out=us_c[:], in_=stU[:])
                    stU = us_c
                    zu_ps = ps1.tile([2, BC], FP, tag="pmisc")
                    nc.tensor.matmul(out=zu_ps[:], lhsT=uones[:], rhs=stU[:], start=True, stop=True)
                    zsl = zbufU[:, (nU // R) * BC : (nU // R + 1) * BC]
                    nc.vector.tensor_copy(out=zsl, in_=zu_ps[:])
                    rzu = sb.tile([2, BC], FP, tag="rzu")
                    nc.vector.reciprocal(out=rzu[:], in_=zu_ps[:])
                    rzu_b = sb.tile([2, BC], BF, tag="rzu_b")
                    nc.vector.tensor_copy(out=rzu_b[:], in_=rzu[:])
                    rzu_rep = ps1.tile([UROW, BC], FP, tag="pmisc")
                    nc.tensor.matmul(out=rzu_rep[:], lhsT=usel[:], rhs=rzu_b[:], start=True, stop=True)
                    rzu_s = sb.tile([UROW, BC], BF, tag="rzu_s")
                    nc.vector.tensor_copy(out=rzu_s[:], in_=rzu_rep[:])
                    us_sc = sb.tile([UROW, BC], BF, tag="us_s")
                    nc.vector.tensor_tensor(out=us_sc[:], in0=stU[:], in1=rzu_s[:], op=ALU.mult)
                    stU = us_sc

        # ---------------- stitch: alpha_pp ----------------
        # stage 1: y1 = S_2^T r  (chunk k=1, rhs = bwd result), then log-shift
        stY1 = psc.tile([128, GQ], FP, tag="stY1")
        nc.vector.memset(stY1[:], 1.0)
        for b in range(BC):
            l, g0 = b % 3, b // 3
            pb = 32 * l
            g = GQ + g0
            nc.tensor.matmul(
                out=stY1[pb : pb + K, g0 : g0 + 1],
                lhsT=intS[pb : pb + K, g * K : (g + 1) * K],
                rhs=endS[pb : pb + K, GQ + g0 : GQ + g0 + 1],
                start=True, stop=True,
            )
        lnY1 = sb.tile([128, GQ], FP, tag="lnY1")
        nc.scalar.activation(out=lnY1[:], in_=stY1[:], func=AF.Ln)
        y1 = sb.tile([128, GQ], BF, tag="y1")
        b1t = sb.tile([128, 1], FP, tag="b1t")
        nc.vector.memset(b1t[:], BIAS1)
        nc.scalar.activation(out=y1[:], in_=lnY1[:], func=AF.Exp, bias=b1t[:, :])

        # stage 2: y2 = S_1^T y1
        stY2 = psc.tile([128, GQ], FP, tag="stY2")
        nc.vector.memset(stY2[:], 1.0)
        for b in range(BC):
            l, g0 = b % 3, b // 3
            pb = 32 * l
            nc.tensor.matmul(
                out=stY2[pb : pb + K, g0 : g0 + 1],
                lhsT=intS[pb : pb + K, g0 * K : (g0 + 1) * K],
                rhs=y1[pb : pb + K, g0 : g0 + 1],
                start=True, stop=True,
            )
        lnY2 = sb.tile([128, GQ], FP, tag="lnY2")
        nc.scalar.activation(out=lnY2[:], in_=stY2[:], func=AF.Ln)
        y2 = sb.tile([128, GQ], BF, tag="y2")
        b2t = sb.tile([128, 1], FP, tag="b2t")
        nc.vector.memset(b2t[:], BIAS2)
        nc.scalar.activation(out=y2[:], in_=lnY2[:], func=AF.Exp, bias=b2t[:, :])

        # final: q_b = y2 . v_F   (cross-partition 24-dot via block-ones matmul)
        qp = sb.tile([128, GQ], BF, tag="qp")
        nc.vector.tensor_tensor(out=qp[:], in0=y2[:], in1=endS[:, 0:GQ], op=ALU.mult)
        qps = ps1.tile([3, GQ], FP, tag="pmisc")
        nc.tensor.matmul(out=qps[:], lhsT=blk3[:], rhs=qp[:], start=True, stop=True)
        lnq = sb.tile([3, GQ], FP, tag="lnq")
        nc.scalar.activation(out=lnq[:], in_=qps[:], func=AF.Ln)
        nc.sync.dma_start(
            out=scr[0:1, 0:33].rearrange("o (g l) -> (o l) g", l=3), in_=lnq[:, :]
        )

        # ---------------- unary meet ----------------
        efl = sb.tile([K, BC], FP, tag="efl")
        nc.sync.dma_start(out=efl[:], in_=eflast[:])
        efl_e = sb.tile([K, BC], BF, tag="efl_e")
        nc.scalar.activation(out=efl_e[:], in_=efl[:], func=AF.Exp)
        ustail = sb.tile([UROW, BC], BF, tag="ustail")
        nc.scalar.activation(out=ustail[:], in_=stU[:], func=AF.Copy)
        stU = ustail
        usb_c = sb.tile([K, BC], BF, tag="usb_c")
        nc.sync.dma_start(out=usb_c[:], in_=stU[32 : 32 + K, :])
        um = sb.tile([K, BC], BF, tag="umeet")
        nc.vector.tensor_tensor(out=um[:], in0=stU[0:K, :], in1=usb_c[:], op=ALU.mult)
        nc.vector.tensor_tensor(out=um[:], in0=um[:], in1=efl_e[:], op=ALU.mult)
        ones_k = sb.tile([K, 1], BF, tag="ones_k")
        nc.vector.memset(ones_k[:], 1.0)
        au_ps = ps1.tile([1, BC], FP, tag="pmisc")
        nc.tensor.matmul(out=au_ps[:], lhsT=ones_k[:], rhs=um[:], start=True, stop=True)
        lau = sb.tile([1, BC], FP, tag="lau")
        nc.scalar.activation(out=lau[:], in_=au_ps[:], func=AF.Ln)
        lzU = sb.tile([2, NRN * BC], FP, tag="lzU")
        nc.scalar.activation(out=lzU[:], in_=zbufU[:], func=AF.Ln)
        sU = sb.tile([2, BC], FP, tag="sU")
        nc.vector.tensor_reduce(
            out=sU[:], in_=lzU[:].rearrange("a (s b) -> a b s", b=BC),
            axis=AX.X, op=ALU.add,
        )
        su_ps = ps1.tile([1, BC], FP, tag="pmisc")
        nc.tensor.matmul(out=su_ps[:], lhsT=ones2[:], rhs=sU[:], start=True, stop=True)
        nc.vector.tensor_tensor(out=lau[:], in0=lau[:], in1=su_ps[:], op=ALU.add)
        nc.vector.tensor_scalar(out=lau[:], in0=lau[:], scalar1=CU * (2 * SL), scalar2=None, op0=ALU.add)
        nc.sync.dma_start(out=scr[1:2, 0:32], in_=lau[:])

        # ---------------- final assembly ----------------
        sc = sb.tile([BC, 1], FP, tag="sc")
        nc.vector.tensor_reduce(out=sc[:], in_=gv[:], axis=AX.X, op=ALU.add)
        app = sb.tile([BC, 1], FP, tag="app")
        nc.sync.dma_start(out=app[:], in_=scr[0:1, 0:32].rearrange("o b -> b o"))
        nc.vector.tensor_scalar(
            out=app[:], in0=app[:],
            scalar1=CP * (T) - BIAS1 - BIAS2, scalar2=None, op0=ALU.add,
        )
        alu_ = sb.tile([BC, 1], FP, tag="alu")
        nc.sync.dma_start(out=alu_[:], in_=scr[1:2, 0:32].rearrange("o b -> b o"))
        res = sb.tile([BC, 1], FP, tag="res")
        nc.vector.tensor_tensor(out=res[:], in0=app[:], in1=alu_[:], op=ALU.add)
        nc.vector.tensor_tensor(out=res[:], in0=res[:], in1=sc[:], op=ALU.subtract)
        nc.sync.dma_start(out=nll[:], in_=res[:].rearrange("b o -> (b o)"))

    nc.compile()
    return nc


# ======================= host-side prep =======================

def prep_core_inputs(feats, fpp, transitions, tags, b0, BC, TT):
    H = TT // 2
    fe = feats[b0 : b0 + BC]          # [BC, T, K]
    fp = fpp[b0 : b0 + BC]            # [BC, T, K*K]
    tg = tags[b0 : b0 + BC]           # [BC, T]
    fp4 = fp.reshape(BC, TT, K, K)    # [b, t, n, p]

    NTE = LE // TCS
    NTI = LI // TCS

    # ---- end-chain stream ----
    fppE = np.zeros((NTE, 3, K, GE, TCS, K), np.float32)
    for b in range(BC):
        l, g = b % 3, b // 3
        A = fp4[b, 0:LE].transpose(2, 0, 1).reshape(K, NTE, TCS, K)  # [p, nt, s, n]
        fppE[:, l, :, g, :, :] = A.transpose(1, 0, 2, 3)
        Bm = fp4[b, 510 : 510 - LE : -1]                  # t = 510..319, [LE, n, p]
        Bm = Bm.transpose(1, 0, 2).reshape(K, NTE, TCS, K)  # [n, nt, s, p]
        fppE[:, l, :, GQ + g, :, :] = Bm.transpose(1, 0, 2, 3)

    # ---- interior stream (2 chunks of 64, last step of chunk 2 = identity) ----
    padm = np.full((BC, 1, K, K), -1e9, np.float32)
    for i in range(K):
        padm[:, 0, i, i] = CP
    Cp = np.concatenate([fp4[:, LE : LE + NI * LI - 1].astype(np.float32), padm], axis=1)
    fppI = np.zeros((NTI, 3, K, GI, TCS, K), np.float32)
    for b in range(BC):
        l, g0 = b % 3, b // 3
        for k in range(NI):
            D = Cp[b, LI * k : LI * (k + 1)].transpose(2, 0, 1).reshape(K, NTI, TCS, K)
            fppI[:, l, :, GQ * k + g0, :, :] = D.transpose(1, 0, 2, 3)

    # ---- chain state inits ----
    endS0 = np.zeros((128, GE), np.float32)
    for b in range(BC):
        l, g = b % 3, b // 3
        endS0[32 * l + START, g] = 1.0
        endS0[32 * l : 32 * l + K, GQ + g] = np.exp(fp4[b, TT - 1, STOP, :] - CP)
    identI = np.zeros((128, GI * K), np.float32)
    for l in range(3):
        for g in range(GI):
            identI[32 * l : 32 * l + K, g * K : (g + 1) * K] = np.eye(K)

    # ---- unary Ef table (unchanged from baseline) ----
    ftp2 = np.zeros((SL, UROW, BC), np.float32)
    ftp2[1:, 0:K, :] = fe[:, 0 : H - 1].transpose(1, 2, 0)
    ftp2[:, 32 : 32 + K, :] = fe[:, TT - 1 : H - 1 : -1].transpose(1, 2, 0)
    eflast = np.ascontiguousarray(fe[:, H - 1, :].T, np.float32)

    # ---- gold-path score operands ----
    tgi = np.asarray(tg, np.int64)
    te = np.concatenate([np.full((BC, 1), START, np.int64), tgi,
                         np.full((BC, 1), STOP, np.int64)], axis=1)
    nxt, prv = te[:, 1:], te[:, :-1]
    b_ = np.arange(BC)[:, None]
    t_ = np.arange(TT)[None, :]
    gvals = np.zeros((BC, 3 * TT + 4), np.float32)
    gvals[:, 0 : TT + 1] = transitions[nxt, prv]
    gvals[:, TT + 1 : 2 * TT + 1] = np.take_along_axis(
        fe, tgi[:, :, None], axis=2)[..., 0]
    gvals[:, 2 * TT + 1 : 3 * TT + 1] = fp4[b_, np.minimum(t_, TT - 2),
                                            nxt[:, 0:TT], prv[:, 0:TT]]
    gvals[:, 3 * TT] = fp4[np.arange(BC), TT - 1, STOP, tgi[:, -1]]
    gvals[:, 3 * TT - 1] = fp4[np.arange(BC), TT - 2, nxt[:, TT - 2], prv[:, TT - 2]]

    return {
        "fppE": fppE,
        "fppI": fppI,
        "endS0": endS0.astype(ml_dtypes.bfloat16),
        "identI": identI.astype(ml_dtypes.bfloat16),
        "ftp2": ftp2,
        "eflast": eflast,
        "transT": np.ascontiguousarray(transitions.T, np.float32),
        "transO": np.ascontiguousarray(transitions, np.float32),
        "gvals": gvals,
    }


_NC_CACHE = {}


def get_nc(BC=32, TT=512):
    key = (BC, TT)
    if key not in _NC_CACHE:
        _NC_CACHE[key] = build_kernel(BC=BC)
    return _NC_CACHE[key]


def kernel(feats, feats_pp, transitions, tags):
    feats = np.asarray(feats, np.float32)
    feats_pp = np.asarray(feats_pp, np.float32)
    transitions = np.asarray(transitions, np.float32)
    tags_np = np.asarray(tags)

    BC = B // NCORES
    nc = get_nc(BC, T)
    in_maps = [
        prep_core_inputs(feats, feats_pp, transitions, tags_np, c * BC, BC, T)
        for c in range(NCORES)
    ]
    r = run_bass_kernel_spmd(nc, in_maps, list(range(NCORES)))
    out = np.concatenate([r.results[c]["nll"] for c in range(NCORES)])
    return out.astype(np.float32)


if __name__ == "__main__":
    rng = np.random.default_rng(0)
    feats = rng.standard_normal((B, T, K), dtype=np.float32)
    fpp = rng.standard_normal((B, T, K * K), dtype=np.float32)
    tr = rng.standard_normal((K, K), dtype=np.float32)
    tr[START, :] = -100.0
    tr[:, STOP] = -100.0
    tags = rng.integers(0, K - 2, size=(B, T)).astype(np.int32)
    out = kernel(feats, fpp, tr, tags)
    print(out.shape, out[:4])


# revision 12
# speedup vs baseline: 1.3838x; 1.0061x over previous
"""DTranNER CRF loss kernel for Trainium2 — v2: PE-resident pairwise scan.

Batch (B=256) is sharded 8 ways (32 sentences/core).  The 511-step pairwise
log-semiring scan runs entirely on the tensor engine in factored linear
space, split into four concurrent chain families so no sequential chain
exceeds 192 steps:

* forward vector chain  (t = 0..191):   v <- M_t v,  one [24,24]@[24,1]
  matmul per (sentence, step); lhsT = exp(fpp_t)^T streamed from HBM.
* backward vector chain (t = 510..319): r <- M_t^T r (natural-layout lhsT).
* two interior chunk operators (t = 192..255, 256..318+pad): 24-column
  basis propagation S <- M_t S, one [24,24]@[24,24] matmul per step.
* stitch: alpha_pp = ln( (S_1^T (S_2^T r)) . v ) + const, with constant
  log-shifts between stages (no per-lane renorms needed at these depths).

Lanes are packed 3-per-128-partitions (PE operands must sit at partition
bases 0/32/64); all per-step PSUM->SBUF state copies are batched into 1-2
DVE ops per step.  The fp32 stream is exp'ed in bulk on ACT one tile ahead
of use; interior-chunk stream DMAs issue from the (otherwise idle) GPSIMD
queue so the SP queue serves only the end-chain stream -- the two DMA
pipelines then never head-of-line block each other.  The unary CRF chain
(constant-transition matmuls on PE + DVE multiplies, lazy renorm every 32
slots, all renorm ops on DVE/PE so the ACT exp pipeline is never in its
path) and the gold-path score reduction are as in the DVE baseline.
"""

import numpy as np
import ml_dtypes
from contextlib import ExitStack

import concourse.bass as bass
import concourse.bacc as bacc
import concourse.tile as tile
from concourse import mybir
from concourse.bass_utils import run_bass_kernel_spmd

FP = mybir.dt.float32
BF = mybir.dt.bfloat16

B, T, K = 256, 512, 24
START, STOP = 22, 23
NCORES = 8

AF = mybir.ActivationFunctionType
ALU = mybir.AluOpType
AX = mybir.AxisListType

# chain partition of the 511 chain matrices (t = 0..510)
LE = 192          # fwd covers t [0,192), bwd covers t [319,511) descending
NI = 2            # interior chunks
LI = 64           # interior chunk length (incl. 1 identity pad step)
GQ = 11           # lane column-blocks per 32-sentence family (ceil(32/3))
GE = 2 * GQ       # end-chain column blocks (fwd 0..10, bwd 11..21)
GI = NI * GQ      # interior column blocks
TCS = 4           # steps per streamed tile
CP = 3.8          # exp pre-scale: matrices enter as exp(x - CP)
CU = 3.8          # unary exp pre-scale
BIAS1 = 30.0      # stitch stage-1 log-shift
BIAS2 = 8.0       # stitch stage-2 log-shift
UROW = 64
SL = 256          # unary slots (fwd+bwd packed)
R = 64            # unary lazy-renorm cadence


BUFS_BIG = 2
BUFS_EBIG = 2
PACE_NS = 0
DO_UNARY = True
DO_PAIR = True
DO_ENDS = True
DO_INT = True


def build_kernel(BC=32):
    assert BC == 32
    NTE = LE // TCS          # 24 end steptiles
    NTI = LI // TCS          # 8 interior steptiles
    CWE = GE * TCS * K       # 4224 cols per end steptile
    CWI = GI * TCS * K       # 4224 cols per interior steptile

    nc = bacc.Bacc("TRN2", target_bir_lowering=False)
    fppE = nc.dram_tensor("fppE", [NTE, 3, K, GE, TCS, K], FP, kind="ExternalInput")
    fppI = nc.dram_tensor("fppI", [NTI, 3, K, GI, TCS, K], FP, kind="ExternalInput")
    endS0 = nc.dram_tensor("endS0", [128, GE], BF, kind="ExternalInput")
    identI = nc.dram_tensor("identI", [128, GI * K], BF, kind="ExternalInput")
    ftp2 = nc.dram_tensor("ftp2", [SL, UROW, BC], FP, kind="ExternalInput")
    eflast = nc.dram_tensor("eflast", [K, BC], FP, kind="ExternalInput")
    transT = nc.dram_tensor("transT", [K, K], FP, kind="ExternalInput")
    transO = nc.dram_tensor("transO", [K, K], FP, kind="ExternalInput")
    gvals = nc.dram_tensor("gvals", [BC, 3 * T + 4], FP, kind="ExternalInput")
    nll = nc.dram_tensor("nll", [BC], FP, kind="ExternalOutput")
    scr = nc.dram_tensor("scratch", [4, 40], FP)

    with tile.TileContext(nc) as tc, ExitStack() as ctx:
        sb = ctx.enter_context(tc.tile_pool(name="sb", bufs=3))
        ps2 = ctx.enter_context(tc.tile_pool(name="ps2", bufs=2, space="PSUM"))
        ps1 = ctx.enter_context(tc.tile_pool(name="ps1", bufs=1, space="PSUM"))
        psc = ctx.enter_context(tc.tile_pool(name="psc", bufs=1, space="PSUM"))
        big = ctx.enter_context(tc.tile_pool(name="big", bufs=BUFS_BIG))
        bigE = ctx.enter_context(tc.tile_pool(name="bigE", bufs=4))
        ebigE = ctx.enter_context(tc.tile_pool(name="ebigE", bufs=3))
        ebig = ctx.enter_context(tc.tile_pool(name="ebig", bufs=BUFS_EBIG))
        per = ctx.enter_context(tc.tile_pool(name="per", bufs=1))

        # ---------------- constants ----------------
        cpb = per.tile([128, 1], FP, tag="cpb")
        nc.vector.memset(cpb[:], -CP)
        cub = per.tile([128, 1], FP, tag="cub")
        nc.vector.memset(cub[:], -CU)

        # unary stationary weights (block matrix, fwd rows 0..K / bwd 32..32+K)
        uwst1 = per.tile([K, K], FP, tag="uwst1")
        nc.sync.dma_start(out=uwst1[:], in_=transT[:])
        uwst2 = per.tile([UROW, K], FP, tag="uwst2")
        nc.sync.dma_start(out=uwst2[32 : 32 + K, :], in_=transO[:])
        uw = per.tile([UROW, UROW], BF, tag="uw")
        nc.vector.memset(uw[:], 0.0)
        nc.scalar.activation(out=uw[0:K, 0:K], in_=uwst1[:], func=AF.Exp)
        nc.scalar.activation(
            out=uw[32 : 32 + K, 32 : 32 + K], in_=uwst2[32 : 32 + K, :], func=AF.Exp
        )
        uones = per.tile([UROW, 2], BF, tag="uones")
        nc.vector.memset(uones[:], 0.0)
        nc.vector.memset(uones[0:K, 0:1], 1.0)
        nc.vector.memset(uones[32 : 32 + K, 1:2], 1.0)
        usel = per.tile([2, UROW], BF, tag="usel")
        nc.vector.memset(usel[:], 0.0)
        nc.vector.memset(usel[0:1, 0:K], 1.0)
        rowB = sb.tile([1, UROW], BF, tag="rowB")
        nc.vector.memset(rowB[:], 0.0)
        nc.vector.memset(rowB[0:1, 32 : 32 + K], 1.0)
        nc.sync.dma_start(out=usel[1:2, :], in_=rowB[:])
        ones2 = per.tile([2, 1], FP, tag="ones2")
        nc.vector.memset(ones2[:], 1.0)
        # quadrant block-ones [128, 3] for the final cross-partition dot
        blk3 = per.tile([128, 3], BF, tag="blk3")
        nc.vector.memset(blk3[:], 0.0)
        for l in range(3):
            nc.vector.memset(blk3[32 * l : 32 * l + K, l : l + 1], 1.0)

        # ---------------- chain states ----------------
        endS = per.tile([128, GE], BF, tag="endS")
        nc.vector.memset(endS[:], 0.0)
        nc.sync.dma_start(out=endS[:], in_=endS0[:])
        intS = per.tile([128, GI * K], BF, tag="intS")
        nc.vector.memset(intS[:], 0.0)
        nc.sync.dma_start(out=intS[:], in_=identI[:])

        endPS = psc.tile([128, GE], FP, tag="endPS")
        nc.vector.memset(endPS[:], 0.0)
        intPSA = psc.tile([128, GQ * K], FP, tag="intPSA")
        nc.vector.memset(intPSA[:], 1.0)
        intPSB = psc.tile([128, GQ * K], FP, tag="intPSB")
        nc.vector.memset(intPSB[:], 1.0)

        tc.strict_bb_all_engine_barrier()

        # ---------------- unary Ef table ----------------
        eft = per.tile([UROW, SL * BC], BF, tag="eft")
        cs2 = SL // 4
        cstep = cs2 * BC
        src = ftp2[:, :, :].rearrange("s r j -> r s j")
        for c in range(4):
            ftile = big.tile([UROW, cstep], FP, tag="ftp_in")
            nc.sync.dma_start(
                out=ftile[:].rearrange("p (s j) -> p s j", j=BC),
                in_=src[:, c * cs2 : (c + 1) * cs2, :],
            )
            nc.scalar.activation(
                out=eft[:, c * cstep : (c + 1) * cstep], in_=ftile[:], func=AF.Exp,
                bias=cub[0:UROW, :],
            )

        # unary state [UROW, BC]
        us0 = per.tile([UROW, BC], BF, tag="us0")
        nc.vector.memset(us0[:], 0.0)
        row1 = sb.tile([1, BC], BF, tag="row1")
        nc.vector.memset(row1[:], 1.0)
        nc.sync.dma_start(out=us0[START : START + 1, :], in_=row1[:])
        tstop = sb.tile([UROW, 1], FP, tag="tstop")
        nc.sync.dma_start(
            out=tstop[32 : 32 + K, :],
            in_=transO[STOP : STOP + 1, :].rearrange("o k -> k o"),
        )
        tstop_e = sb.tile([UROW, 1], BF, tag="tstop_e")
        nc.scalar.activation(out=tstop_e[32 : 32 + K, :], in_=tstop[32 : 32 + K, :], func=AF.Exp)
        nc.vector.tensor_copy(
            out=us0[32 : 32 + K, :], in_=tstop_e[32 : 32 + K, :].broadcast_to([K, BC])
        )
        stU = us0
        NRN = SL // R + 1
        zbufU = per.tile([2, NRN * BC], FP, tag="zbufU")
        nc.vector.memset(zbufU[:], 1.0)

        # gold-path score operands
        gv = per.tile([BC, 3 * T + 4], FP, tag="gv")
        nc.sync.dma_start(out=gv[:], in_=gvals[:])

        tc.strict_bb_all_engine_barrier()

        # ---------------- streamed prefetch helpers ----------------
        def load_end(nt):
            st = bigE.tile([128, CWE], FP, tag="stageE")
            for l in range(3):
                nc.sync.dma_start(
                    out=st[32 * l : 32 * l + K, :].rearrange(
                        "p (g s n) -> p g s n", g=GE, s=TCS
                    ),
                    in_=fppE[nt, l],
                )
            return st

        def load_int(nt):
            st = big.tile([128, CWI], FP, tag="stageI")
            for l in range(3):
                nc.gpsimd.dma_start(
                    out=st[32 * l : 32 * l + K, :].rearrange(
                        "p (g s n) -> p g s n", g=GI, s=TCS
                    ),
                    in_=fppI[nt, l],
                )
            return st

        # Software pipeline: at the boundary of tile nt, tile nt+1 is already
        # exp'ed and tile nt+2's DMA is in flight — the matmul stream never
        # waits on ACT or HBM.
        def exp_tile(stage, w, tag):
            pool = ebigE if tag == "expE" else ebig
            e = pool.tile([128, w], BF, tag=tag)
            nc.scalar.activation(out=e[:], in_=stage[:], func=AF.Exp, bias=cpb[:, :])
            return e

        stq = [load_end(0)]
        stageI_t = load_int(0)
        expE = exp_tile(stq.pop(0), CWE, "expE")
        expI = exp_tile(stageI_t, CWI, "expI")
        stq.append(load_end(1))
        stq.append(load_end(2))
        stq.append(load_end(3))
        stageI_t = load_int(1)
        expE_nxt = expI_nxt = None
        nU = 0

        # ---------------- main loop ----------------
        for s in range(LE):
            if PACE_NS:
                tc.tile_set_cur_wait(s * PACE_NS * 1e-6)
            if s % TCS == 0:
                nt = s // TCS
                if expE_nxt is not None:
                    expE = expE_nxt
                if nt + 1 < NTE:
                    expE_nxt = exp_tile(stq.pop(0), CWE, "expE")
                    if nt + 4 < NTE:
                        stq.append(load_end(nt + 4))
            if s % 3 == 0 and s // 3 < LI and (s // 3) % TCS == 0:
                if expI_nxt is not None:
                    expI = expI_nxt
            if s % (3 * TCS) == (3 * TCS) // 2:
                j = s // (3 * TCS)
                if j + 1 < NTI:
                    expI_nxt = exp_tile(stageI_t, CWI, "expI")
                    if j + 2 < NTI:
                        stageI_t = load_int(j + 2)

            # ---- PE: interior chunk matmuls (every 3rd step) ----
            if DO_PAIR and s % 3 == 0 and s // 3 < LI:
                i_s = s // 3
                so = i_s % TCS
                for k in range(NI):
                    for b in range(BC):
                        l, g0 = b % 3, b // 3
                        g = GQ * k + g0
                        pb = 32 * l
                        lhsT = expI[pb : pb + K, (g * TCS + so) * K : (g * TCS + so + 1) * K]
                        if g < GQ:
                            dst = intPSA
                            co = g * K
                        else:
                            dst = intPSB
                            co = (g - GQ) * K
                        nc.tensor.matmul(
                            out=dst[pb : pb + K, co : co + K],
                            lhsT=lhsT,
                            rhs=intS[pb : pb + K, g * K : (g + 1) * K],
                            start=True, stop=True,
                        )

            # ---- PE: end-chain matmuls ----
            so = s % TCS
            for b in (range(BC) if (DO_PAIR and DO_ENDS) else []):
                l, g0 = b % 3, b // 3
                pb = 32 * l
                lhsT = expE[pb : pb + K, (g0 * TCS + so) * K : (g0 * TCS + so + 1) * K]
                nc.tensor.matmul(
                    out=endPS[pb : pb + K, g0 : g0 + 1], lhsT=lhsT,
                    rhs=endS[pb : pb + K, g0 : g0 + 1], start=True, stop=True,
                )
                gB = GQ + g0
                lhsT2 = expE[pb : pb + K, (gB * TCS + so) * K : (gB * TCS + so + 1) * K]
                nc.tensor.matmul(
                    out=endPS[pb : pb + K, gB : gB + 1], lhsT=lhsT2,
                    rhs=endS[pb : pb + K, gB : gB + 1], start=True, stop=True,
                )

            # ---- state copies (PSUM -> SBUF bf16), ahead of the unary ops in
            # the DVE stream so the pairwise chains never queue behind them --
            if DO_PAIR and DO_INT and s % 3 == 0 and s // 3 < LI:
                nc.vector.tensor_copy(out=intS[:, 0 : GQ * K], in_=intPSA[:])
                nc.vector.tensor_copy(out=intS[:, GQ * K : GI * K], in_=intPSB[:])
            if DO_PAIR and DO_ENDS:
                nc.vector.tensor_copy(out=endS[:], in_=endPS[:])

            # ---- unary slots (DVE + PE stationary matmul); capped at one
            # per step so the unary chain never back-pressures the PE queue --
            tgt = min(s + 1, SL) if DO_UNARY else 0
            while nU < tgt:
                g = nU
                ef_sl = eft[:, g * BC : (g + 1) * BC]
                us_m = sb.tile([UROW, BC], BF, tag="us_m")
                nc.vector.tensor_tensor(out=us_m[:], in0=stU[:], in1=ef_sl, op=ALU.mult)
                vu_ps = ps2.tile([UROW, BC], FP, tag="vu")
                nc.tensor.matmul(out=vu_ps[:], lhsT=uw[:], rhs=us_m[:], start=True, stop=True)
                stU = vu_ps
                nU += 1
                if nU % R == 0 and nU < SL:
                    us_c = sb.tile([UROW, BC], BF, tag="us_c")
                    nc.vector.tensor_copy(out=us_c[:], in_=stU[:])
                    stU = us_c
                    zu_ps = ps1.tile([2, BC], FP, tag="pmisc")
                    nc.tensor.matmul(out=zu_ps[:], lhsT=uones[:], rhs=stU[:], start=True, stop=True)
                    zsl = zbufU[:, (nU // R) * BC : (nU // R + 1) * BC]
                    nc.vector.tensor_copy(out=zsl, in_=zu_ps[:])
                    rzu = sb.tile([2, BC], FP, tag="rzu")
                    nc.vector.reciprocal(out=rzu[:], in_=zu_ps[:])
                    rzu_b = sb.tile([2, BC], BF, tag="rzu_b")
                    nc.vector.tensor_copy(out=rzu_b[:], in_=rzu[:])
                    rzu_rep = ps1.tile([UROW, BC], FP, tag="pmisc")
                    nc.tensor.matmul(out=rzu_rep[:], lhsT=usel[:], rhs=rzu_b[:], start=True, stop=True)
                    rzu_s = sb.tile([UROW, BC], BF, tag="rzu_s")
                    nc.vector.tensor_copy(out=rzu_s[:], in_=rzu_rep[:])
                    us_sc = sb.tile([UROW, BC], BF, tag="us_s")
                    nc.vector.tensor_tensor(out=us_sc[:], in0=stU[:], in1=rzu_s[:], op=ALU.mult)
                    stU = us_sc

        # ---- drain remaining unary slots ----
        while nU < (SL if DO_UNARY else 0):
            g = nU
            ef_sl = eft[:, g * BC : (g + 1) * BC]
            us_m = sb.tile([UROW, BC], BF, tag="us_m")
            nc.vector.tensor_tensor(out=us_m[:], in0=stU[:], in1=ef_sl, op=ALU.mult)
            vu_ps = ps2.tile([UROW, BC], FP, tag="vu")
            nc.tensor.matmul(out=vu_ps[:], lhsT=uw[:], rhs=us_m[:], start=True, stop=True)
            stU = vu_ps
            nU += 1
            if nU % R == 0 and nU < SL:
                us_c = sb.tile([UROW, BC], BF, tag="us_c")
                nc.vector.tensor_copy(out=us_c[:], in_=stU[:])
                stU = us_c
                zu_ps = ps1.tile([2, BC], FP, tag="pmisc")
                nc.tensor.matmul(out=zu_ps[:], lhsT=uones[:], rhs=stU[:], start=True, stop=True)
                zsl = zbufU[:, (nU // R) * BC : (nU // R + 1) * BC]
                nc.vector.tensor_copy(out=zsl, in_=zu_ps[:])
                rzu = sb.tile([2, BC], FP, tag="rzu")
                nc.vector.reciprocal(out=rzu[:], in_=zu_ps[:])
                rzu_b = sb.tile([2, BC], BF, tag="rzu_b")
                nc.vector.tensor_copy(out=rzu_b[:], in_=rzu[:])
                rzu_rep = ps1.tile([UROW, BC], FP, tag="pmisc")
                nc.tensor.matmul(out=rzu_rep[:], lhsT=usel[:], rhs=rzu_b[:], start=True, stop=True)
                rzu_s = sb.tile([UROW, BC], BF, tag="rzu_s")
                nc.vector.tensor_copy(out=rzu_s[:], in_=rzu_rep[:])
                us_sc = sb.tile([UROW, BC], BF, tag="us_s")
                nc.vector.tensor_tensor(out=us_sc[:], in0=stU[:], in1=rzu_s[:], op=ALU.mult)
                stU = us_sc

        # ---------------- stitch: alpha_pp ----------------
        # stage 1: y1 = S_2^T r  (chunk k=1, rhs = bwd result), then log-shift
        stY1 = psc.tile([128, GQ], FP, tag="stY1")
        nc.vector.memset(stY1[:], 1.0)
        for b in range(BC):
            l, g0 = b % 3, b // 3
            pb = 32 * l
            g = GQ + g0
            nc.tensor.matmul(
                out=stY1[pb : pb + K, g0 : g0 + 1],
                lhsT=intS[pb : pb + K, g * K : (g + 1) * K],
                rhs=endS[pb : pb + K, GQ + g0 : GQ + g0 + 1],
                start=True, stop=True,
            )
        lnY1 = sb.tile([128, GQ], FP, tag="lnY1")
        nc.scalar.activation(out=lnY1[:], in_=stY1[:], func=AF.Ln)
        y1 = sb.tile([128, GQ], BF, tag="y1")
        b1t = sb.tile([128, 1], FP, tag="b1t")
        nc.vector.memset(b1t[:], BIAS1)
        nc.scalar.activation(out=y1[:], in_=lnY1[:], func=AF.Exp, bias=b1t[:, :])

        # stage 2: y2 = S_1^T y1
        stY2 = psc.tile([128, GQ], FP, tag="stY2")
        nc.vector.memset(stY2[:], 1.0)
        for b in range(BC):
            l, g0 = b % 3, b // 3
            pb = 32 * l
            nc.tensor.matmul(
                out=stY2[pb : pb + K, g0 : g0 + 1],
                lhsT=intS[pb : pb + K, g0 * K : (g0 + 1) * K],
                rhs=y1[pb : pb + K, g0 : g0 + 1],
                start=True, stop=True,
            )
        lnY2 = sb.tile([128, GQ], FP, tag="lnY2")
        nc.scalar.activation(out=lnY2[:], in_=stY2[:], func=AF.Ln)
        y2 = sb.tile([128, GQ], BF, tag="y2")
        b2t = sb.tile([128, 1], FP, tag="b2t")
        nc.vector.memset(b2t[:], BIAS2)
        nc.scalar.activation(out=y2[:], in_=lnY2[:], func=AF.Exp, bias=b2t[:, :])

        # final: q_b = y2 . v_F   (cross-partition 24-dot via block-ones matmul)
        qp = sb.tile([128, GQ], BF, tag="qp")
        nc.vector.tensor_tensor(out=qp[:], in0=y2[:], in1=endS[:, 0:GQ], op=ALU.mult)
        qps = ps1.tile([3, GQ], FP, tag="pmisc")
        nc.tensor.matmul(out=qps[:], lhsT=blk3[:], rhs=qp[:], start=True, stop=True)
        lnq = sb.tile([3, GQ], FP, tag="lnq")
        nc.scalar.activation(out=lnq[:], in_=qps[:], func=AF.Ln)
        nc.sync.dma_start(
            out=scr[0:1, 0:33].rearrange("o (g l) -> (o l) g", l=3), in_=lnq[:, :]
        )

        # ---------------- unary meet ----------------
        efl = sb.tile([K, BC], FP, tag="efl")
        nc.sync.dma_start(out=efl[:], in_=eflast[:])
        efl_e = sb.tile([K, BC], BF, tag="efl_e")
        nc.scalar.activation(out=efl_e[:], in_=efl[:], func=AF.Exp)
        ustail = sb.tile([UROW, BC], BF, tag="ustail")
        nc.scalar.activation(out=ustail[:], in_=stU[:], func=AF.Copy)
        stU = ustail
        usb_c = sb.tile([K, BC], BF, tag="usb_c")
        nc.sync.dma_start(out=usb_c[:], in_=stU[32 : 32 + K, :])
        um = sb.tile([K, BC], BF, tag="umeet")
        nc.vector.tensor_tensor(out=um[:], in0=stU[0:K, :], in1=usb_c[:], op=ALU.mult)
        nc.vector.tensor_tensor(out=um[:], in0=um[:], in1=efl_e[:], op=ALU.mult)
        ones_k = sb.tile([K, 1], BF, tag="ones_k")
        nc.vector.memset(ones_k[:], 1.0)
        au_ps = ps1.tile([1, BC], FP, tag="pmisc")
        nc.tensor.matmul(out=au_ps[:], lhsT=ones_k[:], rhs=um[:], start=True, stop=True)
        lau = sb.tile([1, BC], FP, tag="lau")
        nc.scalar.activation(out=lau[:], in_=au_ps[:], func=AF.Ln)
        lzU = sb.tile([2, NRN * BC], FP, tag="lzU")
        nc.scalar.activation(out=lzU[:], in_=zbufU[:], func=AF.Ln)
        sU = sb.tile([2, BC], FP, tag="sU")
        nc.vector.tensor_reduce(
            out=sU[:], in_=lzU[:].rearrange("a (s b) -> a b s", b=BC),
            axis=AX.X, op=ALU.add,
        )
        su_ps = ps1.tile([1, BC], FP, tag="pmisc")
        nc.tensor.matmul(out=su_ps[:], lhsT=ones2[:], rhs=sU[:], start=True, stop=True)
        nc.vector.tensor_tensor(out=lau[:], in0=lau[:], in1=su_ps[:], op=ALU.add)
        nc.vector.tensor_scalar(out=lau[:], in0=lau[:], scalar1=CU * (2 * SL), scalar2=None, op0=ALU.add)
        nc.sync.dma_start(out=scr[1:2, 0:32], in_=lau[:])

        # ---------------- final assembly ----------------
        sc = sb.tile([BC, 1], FP, tag="sc")
        nc.vector.tensor_reduce(out=sc[:], in_=gv[:], axis=AX.X, op=ALU.add)
        app = sb.tile([BC, 1], FP, tag="app")
        nc.sync.dma_start(out=app[:], in_=scr[0:1, 0:32].rearrange("o b -> b o"))
        nc.vector.tensor_scalar(
            out=app[:], in0=app[:],
            scalar1=CP * (T) - BIAS1 - BIAS2, scalar2=None, op0=ALU.add,
        )
        alu_ = sb.tile([BC, 1], FP, tag="alu")
        nc.sync.dma_start(out=alu_[:], in_=scr[1:2, 0:32].rearrange("o b -> b o"))
        res = sb.tile([BC, 1], FP, tag="res")
        nc.vector.tensor_tensor(out=res[:], in0=app[:], in1=alu_[:], op=ALU.add)
        nc.vector.tensor_tensor(out=res[:], in0=res[:], in1=sc[:], op=ALU.subtract)
        nc.sync.dma_start(out=nll[:], in_=res[:].rearrange("b o -> (b o)"))

    nc.compile()
    return nc


# ======================= host-side prep =======================

def prep_core_inputs(feats, fpp, transitions, tags, b0, BC, TT):
    H = TT // 2
    fe = feats[b0 : b0 + BC]          # [BC, T, K]
    fp = fpp[b0 : b0 + BC]            # [BC, T, K*K]
    tg = tags[b0 : b0 + BC]           # [BC, T]
    fp4 = fp.reshape(BC, TT, K, K)    # [b, t, n, p]

    NTE = LE // TCS
    NTI = LI // TCS

    # ---- end-chain stream ----
    fppE = np.zeros((NTE, 3, K, GE, TCS, K), np.float32)
    for b in range(BC):
        l, g = b % 3, b // 3
        A = fp4[b, 0:LE].transpose(2, 0, 1).reshape(K, NTE, TCS, K)  # [p, nt, s, n]
        fppE[:, l, :, g, :, :] = A.transpose(1, 0, 2, 3)
        Bm = fp4[b, 510 : 510 - LE : -1]                  # t = 510..319, [LE, n, p]
        Bm = Bm.transpose(1, 0, 2).reshape(K, NTE, TCS, K)  # [n, nt, s, p]
        fppE[:, l, :, GQ + g, :, :] = Bm.transpose(1, 0, 2, 3)

    # ---- interior stream (2 chunks of 64, last step of chunk 2 = identity) ----
    padm = np.full((BC, 1, K, K), -1e9, np.float32)
    for i in range(K):
        padm[:, 0, i, i] = CP
    Cp = np.concatenate([fp4[:, LE : LE + NI * LI - 1].astype(np.float32), padm], axis=1)
    fppI = np.zeros((NTI, 3, K, GI, TCS, K), np.float32)
    for b in range(BC):
        l, g0 = b % 3, b // 3
        for k in range(NI):
            D = Cp[b, LI * k : LI * (k + 1)].transpose(2, 0, 1).reshape(K, NTI, TCS, K)
            fppI[:, l, :, GQ * k + g0, :, :] = D.transpose(1, 0, 2, 3)

    # ---- chain state inits ----
    endS0 = np.zeros((128, GE), np.float32)
    for b in range(BC):
        l, g = b % 3, b // 3
        endS0[32 * l + START, g] = 1.0
        endS0[32 * l : 32 * l + K, GQ + g] = np.exp(fp4[b, TT - 1, STOP, :] - CP)
    identI = np.zeros((128, GI * K), np.float32)
    for l in range(3):
        for g in range(GI):
            identI[32 * l : 32 * l + K, g * K : (g + 1) * K] = np.eye(K)

    # ---- unary Ef table (unchanged from baseline) ----
    ftp2 = np.zeros((SL, UROW, BC), np.float32)
    ftp2[1:, 0:K, :] = fe[:, 0 : H - 1].transpose(1, 2, 0)
    ftp2[:, 32 : 32 + K, :] = fe[:, TT - 1 : H - 1 : -1].transpose(1, 2, 0)
    eflast = np.ascontiguousarray(fe[:, H - 1, :].T, np.float32)

    # ---- gold-path score operands ----
    tgi = np.asarray(tg, np.int64)
    te = np.concatenate([np.full((BC, 1), START, np.int64), tgi,
                         np.full((BC, 1), STOP, np.int64)], axis=1)
    nxt, prv = te[:, 1:], te[:, :-1]
    b_ = np.arange(BC)[:, None]
    t_ = np.arange(TT)[None, :]
    gvals = np.zeros((BC, 3 * TT + 4), np.float32)
    gvals[:, 0 : TT + 1] = transitions[nxt, prv]
    gvals[:, TT + 1 : 2 * TT + 1] = np.take_along_axis(
        fe, tgi[:, :, None], axis=2)[..., 0]
    gvals[:, 2 * TT + 1 : 3 * TT + 1] = fp4[b_, np.minimum(t_, TT - 2),
                                            nxt[:, 0:TT], prv[:, 0:TT]]
    gvals[:, 3 * TT] = fp4[np.arange(BC), TT - 1, STOP, tgi[:, -1]]
    gvals[:, 3 * TT - 1] = fp4[np.arange(BC), TT - 2, nxt[:, TT - 2], prv[:, TT - 2]]

    return {
        "fppE": fppE,
        "fppI": fppI,
        "endS0": endS0.astype(ml_dtypes.bfloat16),
        "identI": identI.astype(ml_dtypes.bfloat16),
        "ftp2": ftp2,
        "eflast": eflast,
        "transT": np.ascontiguousarray(transitions.T, np.float32),
        "transO": np.ascontiguousarray(transitions, np.float32),
        "gvals": gvals,
    }


_NC_CACHE = {}


def get_nc(BC=32, TT=512):
    key = (BC, TT)
    if key not in _NC_CACHE:
        _NC_CACHE[key] = build_kernel(BC=BC)
    return _NC_CACHE[key]


def kernel(feats, feats_pp, transitions, tags):
    feats = np.asarray(feats, np.float32)
    feats_pp = np.asarray(feats_pp, np.float32)
    transitions = np.asarray(transitions, np.float32)
    tags_np = np.asarray(tags)

    BC = B // NCORES
    nc = get_nc(BC, T)
    in_maps = [
        prep_core_inputs(feats, feats_pp, transitions, tags_np, c * BC, BC, T)
        for c in range(NCORES)
    ]
    r = run_bass_kernel_spmd(nc, in_maps, list(range(NCORES)))
    out = np.concatenate([r.results[c]["nll"] for c in range(NCORES)])
    return out.astype(np.float32)


if __name__ == "__main__":
    rng = np.random.default_rng(0)
    feats = rng.standard_normal((B, T, K), dtype=np.float32)
    fpp = rng.standard_normal((B, T, K * K), dtype=np.float32)
    tr = rng.standard_normal((K, K), dtype=np.float32)
    tr[START, :] = -100.0
    tr[:, STOP] = -100.0
    tags = rng.integers(0, K - 2, size=(B, T)).astype(np.int32)
    out = kernel(feats, fpp, tr, tags)
    print(out.shape, out[:4])


# revision 21
# speedup vs baseline: 1.4150x; 1.0226x over previous
"""DTranNER CRF loss kernel for Trainium2 — v2: PE-resident pairwise scan.

Batch (B=256) is sharded 8 ways (32 sentences/core).  The 511-step pairwise
log-semiring scan runs entirely on the tensor engine in factored linear
space, split into four concurrent chain families so no sequential chain
exceeds 192 steps:

* forward vector chain  (t = 0..191):   v <- M_t v,  one [24,24]@[24,1]
  matmul per (sentence, step); lhsT = exp(fpp_t)^T streamed from HBM.
* backward vector chain (t = 510..319): r <- M_t^T r (natural-layout lhsT).
* two interior chunk operators (t = 192..255, 256..318+pad): 24-column
  basis propagation S <- M_t S, one [24,24]@[24,24] matmul per step.
* stitch: alpha_pp = ln( (S_1^T (S_2^T r)) . v ) + const, with constant
  log-shifts between stages (no per-lane renorms needed at these depths).

Lanes are packed 3-per-128-partitions (PE operands must sit at partition
bases 0/32/64); all per-step PSUM->SBUF state copies are batched into 1-2
DVE ops per step.  The fp32 stream is exp'ed in bulk on ACT one tile ahead
of use; interior-chunk stream DMAs issue from the (otherwise idle) GPSIMD
queue so the SP queue serves only the end-chain stream -- the two DMA
pipelines then never head-of-line block each other.  The unary CRF chain
(constant-transition matmuls on PE + DVE multiplies, drift-centered
exp pre-scale with a single mid-chain renorm, all renorm ops on DVE/PE so
the ACT exp pipeline is never in its path) and the gold-path score reduction are as in the DVE baseline.
"""

import numpy as np
import ml_dtypes
from contextlib import ExitStack

import concourse.bass as bass
import concourse.bacc as bacc
import concourse.tile as tile
from concourse import mybir
from concourse.bass_utils import run_bass_kernel_spmd

FP = mybir.dt.float32
BF = mybir.dt.bfloat16

B, T, K = 256, 512, 24
START, STOP = 22, 23
NCORES = 8

AF = mybir.ActivationFunctionType
ALU = mybir.AluOpType
AX = mybir.AxisListType

# chain partition of the 511 chain matrices (t = 0..510)
LE = 192          # fwd covers t [0,192), bwd covers t [319,511) descending
NI = 2            # interior chunks
LI = 66           # interior chunk length (incl. identity pad steps)
GQ = 11           # lane column-blocks per 32-sentence family (ceil(32/3))
GE = 2 * GQ       # end-chain column blocks (fwd 0..10, bwd 11..21)
GI = NI * GQ      # interior column blocks
TCS = 3           # steps per streamed tile
CP = 3.8          # exp pre-scale: matrices enter as exp(x - CP)
CU = 4.1          # unary exp pre-scale (centers unary log-drift for R=128)
BIAS1 = 30.0      # stitch stage-1 log-shift
BIAS2 = 8.0       # stitch stage-2 log-shift
UROW = 64
SL = 256          # unary slots (fwd+bwd packed)
R = 128           # unary lazy-renorm cadence


BUFS_BIG = 2
BUFS_EBIG = 2
PACE_NS = 0
DO_UNARY = True
DO_PAIR = True
DO_ENDS = True
DO_INT = True


def build_kernel(BC=32):
    assert BC == 32
    NTE = LE // TCS          # 24 end steptiles
    NTI = LI // TCS          # 8 interior steptiles
    CWE = GE * TCS * K       # 4224 cols per end steptile
    CWI = GI * TCS * K       # 4224 cols per interior steptile

    nc = bacc.Bacc("TRN2", target_bir_lowering=False)
    fppE = nc.dram_tensor("fppE", [NTE, 3, K, GE, TCS, K], FP, kind="ExternalInput")
    fppI = nc.dram_tensor("fppI", [NTI, 3, K, GI, TCS, K], FP, kind="ExternalInput")
    endS0 = nc.dram_tensor("endS0", [128, GE], BF, kind="ExternalInput")
    identI = nc.dram_tensor("identI", [128, GI * K], BF, kind="ExternalInput")
    ftp2 = nc.dram_tensor("ftp2", [SL, UROW, BC], FP, kind="ExternalInput")
    eflast = nc.dram_tensor("eflast", [K, BC], FP, kind="ExternalInput")
    transT = nc.dram_tensor("transT", [K, K], FP, kind="ExternalInput")
    transO = nc.dram_tensor("transO", [K, K], FP, kind="ExternalInput")
    gvals = nc.dram_tensor("gvals", [BC, 3 * T + 4], FP, kind="ExternalInput")
    nll = nc.dram_tensor("nll", [BC], FP, kind="ExternalOutput")
    scr = nc.dram_tensor("scratch", [4, 40], FP)

    with tile.TileContext(nc) as tc, ExitStack() as ctx:
        sb = ctx.enter_context(tc.tile_pool(name="sb", bufs=3))
        ps2 = ctx.enter_context(tc.tile_pool(name="ps2", bufs=2, space="PSUM"))
        ps1 = ctx.enter_context(tc.tile_pool(name="ps1", bufs=1, space="PSUM"))
        psc = ctx.enter_context(tc.tile_pool(name="psc", bufs=1, space="PSUM"))
        big = ctx.enter_context(tc.tile_pool(name="big", bufs=BUFS_BIG))
        bigE = ctx.enter_context(tc.tile_pool(name="bigE", bufs=4))
        ebigE = ctx.enter_context(tc.tile_pool(name="ebigE", bufs=3))
        ebig = ctx.enter_context(tc.tile_pool(name="ebig", bufs=BUFS_EBIG))
        per = ctx.enter_context(tc.tile_pool(name="per", bufs=1))

        # ---------------- constants ----------------
        cpb = per.tile([128, 1], FP, tag="cpb")
        nc.vector.memset(cpb[:], -CP)
        cub = per.tile([128, 1], FP, tag="cub")
        nc.vector.memset(cub[:], -CU)

        # unary stationary weights (block matrix, fwd rows 0..K / bwd 32..32+K)
        uwst1 = per.tile([K, K], FP, tag="uwst1")
        nc.sync.dma_start(out=uwst1[:], in_=transT[:])
        uwst2 = per.tile([UROW, K], FP, tag="uwst2")
        nc.sync.dma_start(out=uwst2[32 : 32 + K, :], in_=transO[:])
        uw = per.tile([UROW, UROW], BF, tag="uw")
        nc.vector.memset(uw[:], 0.0)
        nc.scalar.activation(out=uw[0:K, 0:K], in_=uwst1[:], func=AF.Exp)
        nc.scalar.activation(
            out=uw[32 : 32 + K, 32 : 32 + K], in_=uwst2[32 : 32 + K, :], func=AF.Exp
        )
        uones = per.tile([UROW, 2], BF, tag="uones")
        nc.vector.memset(uones[:], 0.0)
        nc.vector.memset(uones[0:K, 0:1], 1.0)
        nc.vector.memset(uones[32 : 32 + K, 1:2], 1.0)
        usel = per.tile([2, UROW], BF, tag="usel")
        nc.vector.memset(usel[:], 0.0)
        nc.vector.memset(usel[0:1, 0:K], 1.0)
        rowB = sb.tile([1, UROW], BF, tag="rowB")
        nc.vector.memset(rowB[:], 0.0)
        nc.vector.memset(rowB[0:1, 32 : 32 + K], 1.0)
        nc.sync.dma_start(out=usel[1:2, :], in_=rowB[:])
        ones2 = per.tile([2, 1], FP, tag="ones2")
        nc.vector.memset(ones2[:], 1.0)
        # quadrant block-ones [128, 3] for the final cross-partition dot
        blk3 = per.tile([128, 3], BF, tag="blk3")
        nc.vector.memset(blk3[:], 0.0)
        for l in range(3):
            nc.vector.memset(blk3[32 * l : 32 * l + K, l : l + 1], 1.0)

        # ---------------- chain states ----------------
        endS = per.tile([128, GE], BF, tag="endS")
        nc.vector.memset(endS[:], 0.0)
        nc.sync.dma_start(out=endS[:], in_=endS0[:])
        intS = per.tile([128, GI * K], BF, tag="intS")
        nc.vector.memset(intS[:], 0.0)
        nc.sync.dma_start(out=intS[:], in_=identI[:])

        endPS = psc.tile([128, GE], FP, tag="endPS")
        nc.vector.memset(endPS[:], 0.0)
        intPSA = psc.tile([128, GQ * K], FP, tag="intPSA")
        nc.vector.memset(intPSA[:], 1.0)
        intPSB = psc.tile([128, GQ * K], FP, tag="intPSB")
        nc.vector.memset(intPSB[:], 1.0)

        tc.strict_bb_all_engine_barrier()

        # ---------------- unary Ef table ----------------
        eft = per.tile([UROW, SL * BC], BF, tag="eft")
        cs2 = SL // 4
        cstep = cs2 * BC
        src = ftp2[:, :, :].rearrange("s r j -> r s j")
        for c in range(4):
            ftile = big.tile([UROW, cstep], FP, tag="ftp_in")
            nc.sync.dma_start(
                out=ftile[:].rearrange("p (s j) -> p s j", j=BC),
                in_=src[:, c * cs2 : (c + 1) * cs2, :],
            )
            nc.scalar.activation(
                out=eft[:, c * cstep : (c + 1) * cstep], in_=ftile[:], func=AF.Exp,
                bias=cub[0:UROW, :],
            )

        # unary state [UROW, BC]
        us0 = per.tile([UROW, BC], BF, tag="us0")
        nc.vector.memset(us0[:], 0.0)
        row1 = sb.tile([1, BC], BF, tag="row1")
        nc.vector.memset(row1[:], 1.0)
        nc.sync.dma_start(out=us0[START : START + 1, :], in_=row1[:])
        tstop = sb.tile([UROW, 1], FP, tag="tstop")
        nc.sync.dma_start(
            out=tstop[32 : 32 + K, :],
            in_=transO[STOP : STOP + 1, :].rearrange("o k -> k o"),
        )
        tstop_e = sb.tile([UROW, 1], BF, tag="tstop_e")
        nc.scalar.activation(out=tstop_e[32 : 32 + K, :], in_=tstop[32 : 32 + K, :], func=AF.Exp)
        nc.vector.tensor_copy(
            out=us0[32 : 32 + K, :], in_=tstop_e[32 : 32 + K, :].broadcast_to([K, BC])
        )
        stU = us0
        NRN = SL // R + 1
        zbufU = per.tile([2, NRN * BC], FP, tag="zbufU")
        nc.vector.memset(zbufU[:], 1.0)

        # gold-path score operands
        gv = per.tile([BC, 3 * T + 4], FP, tag="gv")
        nc.sync.dma_start(out=gv[:], in_=gvals[:])

        tc.strict_bb_all_engine_barrier()

        # ---------------- streamed prefetch helpers ----------------
        def load_end(nt):
            st = bigE.tile([128, CWE], FP, tag="stageE")
            for l in range(3):
                nc.sync.dma_start(
                    out=st[32 * l : 32 * l + K, :].rearrange(
                        "p (g s n) -> p g s n", g=GE, s=TCS
                    ),
                    in_=fppE[nt, l],
                )
            return st

        def load_int(nt):
            st = big.tile([128, CWI], FP, tag="stageI")
            for l in range(3):
                nc.gpsimd.dma_start(
                    out=st[32 * l : 32 * l + K, :].rearrange(
                        "p (g s n) -> p g s n", g=GI, s=TCS
                    ),
                    in_=fppI[nt, l],
                )
            return st

        # Software pipeline: at the boundary of tile nt, tile nt+1 is already
        # exp'ed and tile nt+2's DMA is in flight — the matmul stream never
        # waits on ACT or HBM.
        def exp_tile(stage, w, tag):
            pool = ebigE if tag == "expE" else ebig
            e = pool.tile([128, w], BF, tag=tag)
            nc.scalar.activation(out=e[:], in_=stage[:], func=AF.Exp, bias=cpb[:, :])
            return e

        stq = [load_end(0)]
        stageI_t = load_int(0)
        expE = exp_tile(stq.pop(0), CWE, "expE")
        expI = exp_tile(stageI_t, CWI, "expI")
        stq.append(load_end(1))
        stq.append(load_end(2))
        stq.append(load_end(3))
        stageI_t = load_int(1)
        expE_nxt = expI_nxt = None
        nU = 0

        # ---------------- main loop ----------------
        for s in range(LE):
            if PACE_NS:
                tc.tile_set_cur_wait(s * PACE_NS * 1e-6)
            if s % TCS == 0:
                nt = s // TCS
                if expE_nxt is not None:
                    expE = expE_nxt
                if nt + 1 < NTE:
                    expE_nxt = exp_tile(stq.pop(0), CWE, "expE")
                    if nt + 4 < NTE:
                        stq.append(load_end(nt + 4))
            if s % 3 == 0 and s // 3 < LI and (s // 3) % TCS == 0:
                if expI_nxt is not None:
                    expI = expI_nxt
            if s % (3 * TCS) == (3 * TCS) // 2:
                j = s // (3 * TCS)
                if j + 1 < NTI:
                    expI_nxt = exp_tile(stageI_t, CWI, "expI")
                    if j + 2 < NTI:
                        stageI_t = load_int(j + 2)

            # ---- PE: interior chunk matmuls (every 3rd step) ----
            if DO_PAIR and s % 3 == 0 and s // 3 < LI:
                i_s = s // 3
                so = i_s % TCS
                for k in range(NI):
                    for b in range(BC):
                        l, g0 = b % 3, b // 3
                        g = GQ * k + g0
                        pb = 32 * l
                        lhsT = expI[pb : pb + K, (g * TCS + so) * K : (g * TCS + so + 1) * K]
                        if g < GQ:
                            dst = intPSA
                            co = g * K
                        else:
                            dst = intPSB
                            co = (g - GQ) * K
                        nc.tensor.matmul(
                            out=dst[pb : pb + K, co : co + K],
                            lhsT=lhsT,
                            rhs=intS[pb : pb + K, g * K : (g + 1) * K],
                            start=True, stop=True,
                        )

            # ---- PE: end-chain matmuls ----
            so = s % TCS
            for b in (range(BC) if (DO_PAIR and DO_ENDS) else []):
                l, g0 = b % 3, b // 3
                pb = 32 * l
                lhsT = expE[pb : pb + K, (g0 * TCS + so) * K : (g0 * TCS + so + 1) * K]
                nc.tensor.matmul(
                    out=endPS[pb : pb + K, g0 : g0 + 1], lhsT=lhsT,
                    rhs=endS[pb : pb + K, g0 : g0 + 1], start=True, stop=True,
                )
                gB = GQ + g0
                lhsT2 = expE[pb : pb + K, (gB * TCS + so) * K : (gB * TCS + so + 1) * K]
                nc.tensor.matmul(
                    out=endPS[pb : pb + K, gB : gB + 1], lhsT=lhsT2,
                    rhs=endS[pb : pb + K, gB : gB + 1], start=True, stop=True,
                )

            # ---- state copies (PSUM -> SBUF bf16), ahead of the unary ops in
            # the DVE stream so the pairwise chains never queue behind them --
            if DO_PAIR and DO_INT and s % 3 == 0 and s // 3 < LI:
                nc.vector.tensor_copy(out=intS[:, 0 : GQ * K], in_=intPSA[:])
                nc.vector.tensor_copy(out=intS[:, GQ * K : GI * K], in_=intPSB[:])
            if DO_PAIR and DO_ENDS:
                nc.vector.tensor_copy(out=endS[:], in_=endPS[:])

            # ---- unary slots (DVE + PE stationary matmul); capped at one
            # per step so the unary chain never back-pressures the PE queue --
            tgt = min(s + 1, SL) if DO_UNARY else 0
            while nU < tgt:
                g = nU
                ef_sl = eft[:, g * BC : (g + 1) * BC]
                us_m = sb.tile([UROW, BC], BF, tag="us_m")
                nc.vector.tensor_tensor(out=us_m[:], in0=stU[:], in1=ef_sl, op=ALU.mult)
                vu_ps = ps2.tile([UROW, BC], FP, tag="vu")
                nc.tensor.matmul(out=vu_ps[:], lhsT=uw[:], rhs=us_m[:], start=True, stop=True)
                stU = vu_ps
                nU += 1
                if nU % R == 0 and nU < SL:
                    us_c = sb.tile([UROW, BC], BF, tag="us_c")
                    nc.vector.tensor_copy(out=us_c[:], in_=stU[:])
                    stU = us_c
                    zu_ps = ps1.tile([2, BC], FP, tag="pmisc")
                    nc.tensor.matmul(out=zu_ps[:], lhsT=uones[:], rhs=stU[:], start=True, stop=True)
                    zsl = zbufU[:, (nU // R) * BC : (nU // R + 1) * BC]
                    nc.vector.tensor_copy(out=zsl, in_=zu_ps[:])
                    rzu = sb.tile([2, BC], FP, tag="rzu")
                    nc.vector.reciprocal(out=rzu[:], in_=zu_ps[:])
                    rzu_b = sb.tile([2, BC], BF, tag="rzu_b")
                    nc.vector.tensor_copy(out=rzu_b[:], in_=rzu[:])
                    rzu_rep = ps1.tile([UROW, BC], FP, tag="pmisc")
                    nc.tensor.matmul(out=rzu_rep[:], lhsT=usel[:], rhs=rzu_b[:], start=True, stop=True)
                    rzu_s = sb.tile([UROW, BC], BF, tag="rzu_s")
                    nc.vector.tensor_copy(out=rzu_s[:], in_=rzu_rep[:])
                    us_sc = sb.tile([UROW, BC], BF, tag="us_s")
                    nc.vector.tensor_tensor(out=us_sc[:], in0=stU[:], in1=rzu_s[:], op=ALU.mult)
                    stU = us_sc

        # ---- drain interior steps beyond the cadence-3 window ----
        for i_s in range(LE // 3, LI):
            so = i_s % TCS
            for k in range(NI):
                for b in range(BC):
                    l, g0 = b % 3, b // 3
                    g = GQ * k + g0
                    pb = 32 * l
                    lhsT = expI[pb : pb + K, (g * TCS + so) * K : (g * TCS + so + 1) * K]
                    if g < GQ:
                        dst = intPSA
                        co = g * K
                    else:
                        dst = intPSB
                        co = (g - GQ) * K
                    nc.tensor.matmul(
                        out=dst[pb : pb + K, co : co + K],
                        lhsT=lhsT,
                        rhs=intS[pb : pb + K, g * K : (g + 1) * K],
                        start=True, stop=True,
                    )
            nc.vector.tensor_copy(out=intS[:, 0 : GQ * K], in_=intPSA[:])
            nc.vector.tensor_copy(out=intS[:, GQ * K : GI * K], in_=intPSB[:])

        # ---- drain remaining unary slots ----
        while nU < (SL if DO_UNARY else 0):
            g = nU
            ef_sl = eft[:, g * BC : (g + 1) * BC]
            us_m = sb.tile([UROW, BC], BF, tag="us_m")
            nc.vector.tensor_tensor(out=us_m[:], in0=stU[:], in1=ef_sl, op=ALU.mult)
            vu_ps = ps2.tile([UROW, BC], FP, tag="vu")
            nc.tensor.matmul(out=vu_ps[:], lhsT=uw[:], rhs=us_m[:], start=True, stop=True)
            stU = vu_ps
            nU += 1
            if nU % R == 0 and nU < SL:
                us_c = sb.tile([UROW, BC], BF, tag="us_c")
                nc.vector.tensor_copy(out=us_c[:], in_=stU[:])
                stU = us_c
                zu_ps = ps1.tile([2, BC], FP, tag="pmisc")
                nc.tensor.matmul(out=zu_ps[:], lhsT=uones[:], rhs=stU[:], start=True, stop=True)
                zsl = zbufU[:, (nU // R) * BC : (nU // R + 1) * BC]
                nc.vector.tensor_copy(out=zsl, in_=zu_ps[:])
                rzu = sb.tile([2, BC], FP, tag="rzu")
                nc.vector.reciprocal(out=rzu[:], in_=zu_ps[:])
                rzu_b = sb.tile([2, BC], BF, tag="rzu_b")
                nc.vector.tensor_copy(out=rzu_b[:], in_=rzu[:])
                rzu_rep = ps1.tile([UROW, BC], FP, tag="pmisc")
                nc.tensor.matmul(out=rzu_rep[:], lhsT=usel[:], rhs=rzu_b[:], start=True, stop=True)
                rzu_s = sb.tile([UROW, BC], BF, tag="rzu_s")
                nc.vector.tensor_copy(out=rzu_s[:], in_=rzu_rep[:])
                us_sc = sb.tile([UROW, BC], BF, tag="us_s")
                nc.vector.tensor_tensor(out=us_sc[:], in0=stU[:], in1=rzu_s[:], op=ALU.mult)
                stU = us_sc

        # ---------------- stitch: alpha_pp ----------------
        # stage 1: y1 = S_2^T r  (chunk k=1, rhs = bwd result), then log-shift
        stY1 = psc.tile([128, GQ], FP, tag="stY1")
        nc.vector.memset(stY1[:], 1.0)
        for b in range(BC):
            l, g0 = b % 3, b // 3
            pb = 32 * l
            g = GQ + g0
            nc.tensor.matmul(
                out=stY1[pb : pb + K, g0 : g0 + 1],
                lhsT=intS[pb : pb + K, g * K : (g + 1) * K],
                rhs=endS[pb : pb + K, GQ + g0 : GQ + g0 + 1],
                start=True, stop=True,
            )
        lnY1 = sb.tile([128, GQ], FP, tag="lnY1")
        nc.scalar.activation(out=lnY1[:], in_=stY1[:], func=AF.Ln)
        y1 = sb.tile([128, GQ], BF, tag="y1")
        b1t = sb.tile([128, 1], FP, tag="b1t")
        nc.vector.memset(b1t[:], BIAS1)
        nc.scalar.activation(out=y1[:], in_=lnY1[:], func=AF.Exp, bias=b1t[:, :])

        # stage 2: y2 = S_1^T y1
        stY2 = psc.tile([128, GQ], FP, tag="stY2")
        nc.vector.memset(stY2[:], 1.0)
        for b in range(BC):
            l, g0 = b % 3, b // 3
            pb = 32 * l
            nc.tensor.matmul(
                out=stY2[pb : pb + K, g0 : g0 + 1],
                lhsT=intS[pb : pb + K, g0 * K : (g0 + 1) * K],
                rhs=y1[pb : pb + K, g0 : g0 + 1],
                start=True, stop=True,
            )
        lnY2 = sb.tile([128, GQ], FP, tag="lnY2")
        nc.scalar.activation(out=lnY2[:], in_=stY2[:], func=AF.Ln)
        y2 = sb.tile([128, GQ], BF, tag="y2")
        b2t = sb.tile([128, 1], FP, tag="b2t")
        nc.vector.memset(b2t[:], BIAS2)
        nc.scalar.activation(out=y2[:], in_=lnY2[:], func=AF.Exp, bias=b2t[:, :])

        # final: q_b = y2 . v_F   (cross-partition 24-dot via block-ones matmul)
        qp = sb.tile([128, GQ], BF, tag="qp")
        nc.vector.tensor_tensor(out=qp[:], in0=y2[:], in1=endS[:, 0:GQ], op=ALU.mult)
        qps = ps1.tile([3, GQ], FP, tag="pmisc")
        nc.tensor.matmul(out=qps[:], lhsT=blk3[:], rhs=qp[:], start=True, stop=True)
        lnq = sb.tile([3, GQ], FP, tag="lnq")
        nc.scalar.activation(out=lnq[:], in_=qps[:], func=AF.Ln)
        nc.sync.dma_start(
            out=scr[0:1, 0:33].rearrange("o (g l) -> (o l) g", l=3), in_=lnq[:, :]
        )

        # ---------------- unary meet ----------------
        efl = sb.tile([K, BC], FP, tag="efl")
        nc.sync.dma_start(out=efl[:], in_=eflast[:])
        efl_e = sb.tile([K, BC], BF, tag="efl_e")
        nc.scalar.activation(out=efl_e[:], in_=efl[:], func=AF.Exp)
        ustail = sb.tile([UROW, BC], BF, tag="ustail")
        nc.scalar.activation(out=ustail[:], in_=stU[:], func=AF.Copy)
        stU = ustail
        usb_c = sb.tile([K, BC], BF, tag="usb_c")
        nc.sync.dma_start(out=usb_c[:], in_=stU[32 : 32 + K, :])
        um = sb.tile([K, BC], BF, tag="umeet")
        nc.vector.tensor_tensor(out=um[:], in0=stU[0:K, :], in1=usb_c[:], op=ALU.mult)
        nc.vector.tensor_tensor(out=um[:], in0=um[:], in1=efl_e[:], op=ALU.mult)
        ones_k = sb.tile([K, 1], BF, tag="ones_k")
        nc.vector.memset(ones_k[:], 1.0)
        au_ps = ps1.tile([1, BC], FP, tag="pmisc")
        nc.tensor.matmul(out=au_ps[:], lhsT=ones_k[:], rhs=um[:], start=True, stop=True)
        lau = sb.tile([1, BC], FP, tag="lau")
        nc.scalar.activation(out=lau[:], in_=au_ps[:], func=AF.Ln)
        lzU = sb.tile([2, NRN * BC], FP, tag="lzU")
        nc.scalar.activation(out=lzU[:], in_=zbufU[:], func=AF.Ln)
        sU = sb.tile([2, BC], FP, tag="sU")
        nc.vector.tensor_reduce(
            out=sU[:], in_=lzU[:].rearrange("a (s b) -> a b s", b=BC),
            axis=AX.X, op=ALU.add,
        )
        su_ps = ps1.tile([1, BC], FP, tag="pmisc")
        nc.tensor.matmul(out=su_ps[:], lhsT=ones2[:], rhs=sU[:], start=True, stop=True)
        nc.vector.tensor_tensor(out=lau[:], in0=lau[:], in1=su_ps[:], op=ALU.add)
        nc.vector.tensor_scalar(out=lau[:], in0=lau[:], scalar1=CU * (2 * SL), scalar2=None, op0=ALU.add)
        nc.sync.dma_start(out=scr[1:2, 0:32], in_=lau[:])

        # ---------------- final assembly ----------------
        sc = sb.tile([BC, 1], FP, tag="sc")
        nc.vector.tensor_reduce(out=sc[:], in_=gv[:], axis=AX.X, op=ALU.add)
        app = sb.tile([BC, 1], FP, tag="app")
        nc.sync.dma_start(out=app[:], in_=scr[0:1, 0:32].rearrange("o b -> b o"))
        nc.vector.tensor_scalar(
            out=app[:], in0=app[:],
            scalar1=CP * (T) - BIAS1 - BIAS2, scalar2=None, op0=ALU.add,
        )
        alu_ = sb.tile([BC, 1], FP, tag="alu")
        nc.sync.dma_start(out=alu_[:], in_=scr[1:2, 0:32].rearrange("o b -> b o"))
        res = sb.tile([BC, 1], FP, tag="res")
        nc.vector.tensor_tensor(out=res[:], in0=app[:], in1=alu_[:], op=ALU.add)
        nc.vector.tensor_tensor(out=res[:], in0=res[:], in1=sc[:], op=ALU.subtract)
        nc.sync.dma_start(out=nll[:], in_=res[:].rearrange("b o -> (b o)"))

    nc.compile()
    return nc


# ======================= host-side prep =======================

def prep_core_inputs(feats, fpp, transitions, tags, b0, BC, TT):
    H = TT // 2
    fe = feats[b0 : b0 + BC]          # [BC, T, K]
    fp = fpp[b0 : b0 + BC]            # [BC, T, K*K]
    tg = tags[b0 : b0 + BC]           # [BC, T]
    fp4 = fp.reshape(BC, TT, K, K)    # [b, t, n, p]

    NTE = LE // TCS
    NTI = LI // TCS

    # ---- end-chain stream ----
    fppE = np.zeros((NTE, 3, K, GE, TCS, K), np.float32)
    for b in range(BC):
        l, g = b % 3, b // 3
        A = fp4[b, 0:LE].transpose(2, 0, 1).reshape(K, NTE, TCS, K)  # [p, nt, s, n]
        fppE[:, l, :, g, :, :] = A.transpose(1, 0, 2, 3)
        Bm = fp4[b, 510 : 510 - LE : -1]                  # t = 510..319, [LE, n, p]
        Bm = Bm.transpose(1, 0, 2).reshape(K, NTE, TCS, K)  # [n, nt, s, p]
        fppE[:, l, :, GQ + g, :, :] = Bm.transpose(1, 0, 2, 3)

    # ---- interior stream (2 chunks of 64, last step of chunk 2 = identity) ----
    npad = NI * LI - (TT - 1 - 2 * LE)
    padm = np.full((BC, npad, K, K), -1e9, np.float32)
    for i in range(K):
        padm[:, :, i, i] = CP
    Cp = np.concatenate(
        [fp4[:, LE : TT - 1 - LE].astype(np.float32), padm], axis=1)
    fppI = np.zeros((NTI, 3, K, GI, TCS, K), np.float32)
    for b in range(BC):
        l, g0 = b % 3, b // 3
        for k in range(NI):
            D = Cp[b, LI * k : LI * (k + 1)].transpose(2, 0, 1).reshape(K, NTI, TCS, K)
            fppI[:, l, :, GQ * k + g0, :, :] = D.transpose(1, 0, 2, 3)

    # ---- chain state inits ----
    endS0 = np.zeros((128, GE), np.float32)
    for b in range(BC):
        l, g = b % 3, b // 3
        endS0[32 * l + START, g] = 1.0
        endS0[32 * l : 32 * l + K, GQ + g] = np.exp(fp4[b, TT - 1, STOP, :] - CP)
    identI = np.zeros((128, GI * K), np.float32)
    for l in range(3):
        for g in range(GI):
            identI[32 * l : 32 * l + K, g * K : (g + 1) * K] = np.eye(K)

    # ---- unary Ef table (unchanged from baseline) ----
    ftp2 = np.zeros((SL, UROW, BC), np.float32)
    ftp2[1:, 0:K, :] = fe[:, 0 : H - 1].transpose(1, 2, 0)
    ftp2[:, 32 : 32 + K, :] = fe[:, TT - 1 : H - 1 : -1].transpose(1, 2, 0)
    eflast = np.ascontiguousarray(fe[:, H - 1, :].T, np.float32)

    # ---- gold-path score operands ----
    tgi = np.asarray(tg, np.int64)
    te = np.concatenate([np.full((BC, 1), START, np.int64), tgi,
                         np.full((BC, 1), STOP, np.int64)], axis=1)
    nxt, prv = te[:, 1:], te[:, :-1]
    b_ = np.arange(BC)[:, None]
    t_ = np.arange(TT)[None, :]
    gvals = np.zeros((BC, 3 * TT + 4), np.float32)
    gvals[:, 0 : TT + 1] = transitions[nxt, prv]
    gvals[:, TT + 1 : 2 * TT + 1] = np.take_along_axis(
        fe, tgi[:, :, None], axis=2)[..., 0]
    gvals[:, 2 * TT + 1 : 3 * TT + 1] = fp4[b_, np.minimum(t_, TT - 2),
                                            nxt[:, 0:TT], prv[:, 0:TT]]
    gvals[:, 3 * TT] = fp4[np.arange(BC), TT - 1, STOP, tgi[:, -1]]
    gvals[:, 3 * TT - 1] = fp4[np.arange(BC), TT - 2, nxt[:, TT - 2], prv[:, TT - 2]]

    return {
        "fppE": fppE,
        "fppI": fppI,
        "endS0": endS0.astype(ml_dtypes.bfloat16),
        "identI": identI.astype(ml_dtypes.bfloat16),
        "ftp2": ftp2,
        "eflast": eflast,
        "transT": np.ascontiguousarray(transitions.T, np.float32),
        "transO": np.ascontiguousarray(transitions, np.float32),
        "gvals": gvals,
    }


_NC_CACHE = {}


def get_nc(BC=32, TT=512):
    key = (BC, TT)
    if key not in _NC_CACHE:
        _NC_CACHE[key] = build_kernel(BC=BC)
    return _NC_CACHE[key]


def kernel(feats, feats_pp, transitions, tags):
    feats = np.asarray(feats, np.float32)
    feats_pp = np.asarray(feats_pp, np.float32)
    transitions = np.asarray(transitions, np.float32)
    tags_np = np.asarray(tags)

    BC = B // NCORES
    nc = get_nc(BC, T)
    in_maps = [
        prep_core_inputs(feats, feats_pp, transitions, tags_np, c * BC, BC, T)
        for c in range(NCORES)
    ]
    r = run_bass_kernel_spmd(nc, in_maps, list(range(NCORES)))
    out = np.concatenate([r.results[c]["nll"] for c in range(NCORES)])
    return out.astype(np.float32)


if __name__ == "__main__":
    rng = np.random.default_rng(0)
    feats = rng.standard_normal((B, T, K), dtype=np.float32)
    fpp = rng.standard_normal((B, T, K * K), dtype=np.float32)
    tr = rng.standard_normal((K, K), dtype=np.float32)
    tr[START, :] = -100.0
    tr[:, STOP] = -100.0
    tags = rng.integers(0, K - 2, size=(B, T)).astype(np.int32)
    out = kernel(feats, fpp, tr, tags)
    print(out.shape, out[:4])
